# revision 75
# baseline (speedup 1.0000x reference)
"""Multi-head causal self-attention (B=2, T=2048, C=1024, H=16) on 8 trn2 cores.

Sharding: data-parallel over batch (2) x tensor-parallel over heads (4 groups
of 4 heads). Core c handles batch b=c//4, head group g=c%4.

Key structure (per core):
  - QKV and output projections run in fp8e4m3 DoubleRow with hi+lo error
    compensation: every operand X is host- (or device-) split into
    X_hi = fp8(X), X_lo = fp8(X - X_hi); each 256-channel contraction uses
    3 DoubleRow matmuls (hi*hi packing two k-tiles, plus one cross-term
    matmul per k-tile computing hi*lo + lo*hi in its two slots) instead of
    2 f16 matmuls -> 4x per-row speed at 3/2 the instruction count = 2.67x,
    with quantization error compensated to ~1e-3. Operands are pre-scaled
    (x*8, W*64, oT*16) to clear the e4m3 subnormal floor; descale happens
    in the PSUM evacuation op.
  - Attention stays f16 in S^T orientation (k on partitions, q free): the
    exp on ACT (~58us) is the attention-phase floor, so cheaper PE matmuls
    there would buy nothing. Causal mask via a PE add-matmul (-200
    strict-upper-tri stationary x identity moving).
  - Rowsums from a ones-column appended to V; recip on DVE (scaled 1/16 so
    oT carries a x16 pre-scale for its fp8 split); partition_broadcast on
    Pool.
  - All projection bias adds are folded into the PSUM evacuation
    (scalar_tensor_tensor with descale scalar + partition-broadcast bias)
    on DVE/Pool -- no bias matmuls on PE.
  - Sub-chunk processing order is sc0, sc1, sc3, sc2: the final
    ReduceScatter is gated by the last chunk's projection, so the smallest
    remaining causal triangle (sc2) goes last; its projection stays f16
    (reads oT directly, skipping the fp8 hi/lo split latency) to keep the
    tail chain short. PSUM is split into 2x[128,1024] S-tile buffers and
    2x[128,512] buffers for QKV/proj units so fillers never contend with
    the attention pipeline; PE warms its p-state on dummy matmuls during
    the DMA-bound preamble.
"""

import os

import numpy as np
import ml_dtypes

import concourse.bacc as bacc
import concourse.mybir as mybir
import concourse.tile as tile
from concourse.bass_utils import run_bass_kernel_spmd

DEBUG = bool(int(os.environ.get("KERNEL_DEBUG", "0")))
PHASE_MARKS = []  # (label, first_instruction_index) for trace attribution

F32 = mybir.dt.float32
F16 = mybir.dt.float16
F8 = mybir.dt.float8e4
E4 = ml_dtypes.float8_e4m3
DR = mybir.MatmulPerfMode.DoubleRow

B, T, C, H = 2, 2048, 1024, 16
HPC = 4                 # heads per core
HD = 64                 # head dim
CG = HPC * 3 * HD       # 768 qkv cols per core
KC = 8                  # contraction chunks (128 channels each)
TT = T // 128           # 16 k tiles
NSC = T // 512          # 4 q sub-chunks
N_CORES = 8
EXP_SCALE = 0.125

XS = 8.0                # host pre-scale on x
WS = 64.0               # host pre-scale on Wqkv / Wproj
OS = 16.0               # device pre-scale on attention output (via recip)
QKV_DESCALE = 1.0 / (XS * WS)
PROJ8_DESCALE = 1.0 / (OS * WS)
PROJ16_DESCALE = 1.0 / OS

# reduce-scatter groups as (row_start, row_end) in COMPLETION order
# (sc0+sc1, then sc3, then sc2 last); each core keeps len/4 rows
RS_GROUPS = [(0, 1024), (1536, 2048), (1024, 1536)]


def _build():
    nc = bacc.Bacc(None, target_bir_lowering=False)

    # x8: [p, w(4), kc(8), e(hi,lo), t(512)] fp8
    x8_in = nc.dram_tensor("x8", [128, 4 * KC * 2 * 512], F8, kind="ExternalInput")
    # w8: [p, kc(8), e(lo,hi), m(768)] fp8
    w8_in = nc.dram_tensor("w8", [128, KC * 2 * CG], F8, kind="ExternalInput")
    qkb_in = nc.dram_tensor("qkb", [128, 4], F32, kind="ExternalInput")
    vb_in = nc.dram_tensor("vb", [1, 256], F16, kind="ExternalInput")
    # wpa8: [p, pair(2), e(lo,hi), c(1024)] fp8 (x64 scaled Wproj rows)
    wpa8_in = nc.dram_tensor("wpa8", [128, 2 * 2 * C], F8, kind="ExternalInput")
    # wpa16: [p, pair(2)*c] f16 unscaled Wproj rows (tail path)
    wpa16_in = nc.dram_tensor("wpa16", [128, 2 * C], F16, kind="ExternalInput")
    # bpr blocks along free dim: [bproj | bproj*OS*WS | bproj*OS]
    # (blocks 1,2 are added in PSUM via a K=1 matmul before a scaled ACT evac)
    bpr_in = nc.dram_tensor("bpr", [1, 3 * C], F16, kind="ExternalInput")
    out_part = nc.dram_tensor("out_part", [T // 4, C], F16, kind="ExternalOutput")

    partial_d = nc.dram_tensor("partial_d", [T, C], F16)
    dbg = {}
    if DEBUG:
        dbg["oT1"] = nc.dram_tensor("dbg_oT1", [128, 2 * 512], F16, kind="ExternalOutput")
        dbg["oT81"] = nc.dram_tensor("dbg_oT81", [128, 4 * 512], F8, kind="ExternalOutput")
        dbg["partial"] = nc.dram_tensor("dbg_partial", [T, C], F16, kind="ExternalOutput")
        dbg["qkT"] = nc.dram_tensor("dbg_qkT", [128, 4 * T], F16, kind="ExternalOutput")
    rsout_d = [
        nc.dram_tensor(f"rsout_d{i}", [(r1 - r0) // 4, C], F16)
        for i, (r0, r1) in enumerate(RS_GROUPS)
    ]

    with tile.TileContext(nc) as tc:
        with (
            tc.tile_pool(name="cpool", bufs=1) as cpool,
            tc.tile_pool(name="main", bufs=1) as main,
            tc.tile_pool(name="stage", bufs=1) as stage,
            tc.tile_pool(name="ps", bufs=1, space="PSUM") as ps,
        ):
            # ---------------- constants ----------------
            vb_bc = cpool.tile([128, 256], F16)
            bias_bc = cpool.tile([128, C], F16)
            ones_col = cpool.tile([1, 128], F16)
            nc.vector.memset(ones_col[:], 1.0)
            # mask stationary: mstat[f, p] = -200 where p > f else 0
            mstat = cpool.tile([128, 128], F16)
            nc.gpsimd.memset(mstat[:], -200.0)
            nc.gpsimd.affine_select(
                out=mstat[:], in_=mstat[:],
                compare_op=mybir.AluOpType.is_ge, fill=0.0,
                base=-1, pattern=[[1, 128]], channel_multiplier=-1,
            )
            # mask moving: identity
            mmov = cpool.tile([128, 128], F16)
            nc.gpsimd.memset(mmov[:], 0.0)
            nc.gpsimd.affine_select(
                out=mmov[:], in_=mmov[:],
                compare_op=mybir.AluOpType.not_equal, fill=1.0,
                base=0, pattern=[[-1, 128]], channel_multiplier=1,
            )

            # ---------------- persistent tensors ----------------
            x8 = main.tile([128, 4 * KC * 2 * 512], F8)
            w8 = main.tile([128, KC * 2 * CG], F8)
            qkb = main.tile([128, 4], F32)
            vb = main.tile([1, 256], F16)
            wpa8 = main.tile([128, 2 * 2 * C], F8)
            wpa16 = main.tile([128, 2 * C], F16)
            bpr = main.tile([1, 3 * C], F16)
            qkT = main.tile([128, 4 * T], F16)             # [Q01;Q23;K01;K23] x T
            v_aug = main.tile([128, TT * HPC * 65], F16)   # per (tt,h): 64 V + ones col
            # f16 attention outs, [pair][512] col layout (x16 pre-scaled);
            # head h lives at partitions 64*(h%2).., column block (h//2)*512
            oT_sb = [
                main.tile([128, 2 * 512], F16, name=f"oT_sb{i}") for i in range(2)
            ]
            # fp8 hi/lo split of oT for the fp8 projection path:
            # [p, pair(2), e(hi,lo), 512]
            oT8_sb = [
                main.tile([128, 2 * 2 * 512], F8, name=f"oT8_sb{i}") for i in range(2)
            ]

            nc.vector.memset(v_aug[:], 1.0)  # ones columns give softmax rowsums

            # ---------------- views ----------------
            w8_r = w8[:].rearrange("p (kc e m) -> p kc e m", kc=KC, e=2)
            w8_in_r = w8_in[:].rearrange("p (kc e m) -> p kc e m", kc=KC, e=2)
            wpa8_r = wpa8[:].rearrange("p (pr e c) -> p pr e c", pr=2, e=2)
            oT8_r = [
                t[:].rearrange("p (pr e c) -> p pr e c", pr=2, e=2) for t in oT8_sb
            ]

            def x8_w(w):
                # [128, kc, e, 512] view of window w
                return x8[:, w * 8192 : (w + 1) * 8192].rearrange(
                    "p (kc e t) -> p kc e t", kc=KC, e=2
                )

            # ---------------- input DMAs ----------------
            # DMA transfers serialize on the DMA-engine pool, so order them
            # by what gates compute: x window 0 + QK weights interleaved per
            # kc pair (attention sc0's S matmuls are the longest dependency
            # chain), V weights next (PV trails S by ~1.5us), then the later
            # x windows.
            for kp in range(4):
                nc.sync.dma_start(
                    x8[:, kp * 2048 : (kp + 1) * 2048],
                    x8_in[:, kp * 2048 : (kp + 1) * 2048],
                )
                nc.sync.dma_start(
                    w8_r[:, 2 * kp : 2 * kp + 2, :, 0:512],
                    w8_in_r[:, 2 * kp : 2 * kp + 2, :, 0:512],
                )
            nc.sync.dma_start(vb[:], vb_in[:])
            nc.sync.dma_start(qkb[:], qkb_in[:])
            nc.gpsimd.partition_broadcast(vb_bc[:], vb[:])
            nc.sync.dma_start(w8_r[:, 0:4, :, 512:768], w8_in_r[:, 0:4, :, 512:768])
            nc.sync.dma_start(w8_r[:, 4:8, :, 512:768], w8_in_r[:, 4:8, :, 512:768])
            nc.sync.dma_start(x8[:, 8192:16384], x8_in[:, 8192:16384])      # w1
            nc.scalar.dma_start(wpa8[:], wpa8_in[:])
            nc.scalar.dma_start(bpr[:], bpr_in[:])
            nc.gpsimd.partition_broadcast(bias_bc[:], bpr[:, 0:C])
            nc.sync.dma_start(x8[:, 16384:24576], x8_in[:, 16384:24576])    # w2
            nc.sync.dma_start(x8[:, 24576:32768], x8_in[:, 24576:32768])    # w3
            nc.scalar.dma_start(wpa16[:], wpa16_in[:])

            # ---------------- emit helpers ----------------
            def mm_pairs(pp_slice, stat_of, mov_of):
                """Emit the compensated fp8 matmul group: for each kc pair,
                hi*hi (2 slots = the 2 k-tiles), then a cross matmul per
                k-tile (slots = hi*lo + lo*hi)."""
                n = KC // 2
                for p in range(n):
                    c0 = 2 * p
                    nc.tensor.matmul(
                        pp_slice, stat_of(c0, None), mov_of(c0, None),
                        start=(p == 0), stop=False, perf_mode=DR,
                    )
                    for c in (c0, c0 + 1):
                        nc.tensor.matmul(
                            pp_slice, stat_of(None, c), mov_of(None, c),
                            start=False, stop=(c == KC - 1), perf_mode=DR,
                        )

            def emit_v(tt):
                PHASE_MARKS.append((f"V{tt}", len(nc.inst_map)))
                w, tloc = divmod(tt, 4)
                xw = x8_w(w)
                tr = slice(tloc * 128, (tloc + 1) * 128)
                pp = ps.tile([128, 512], F32, tag="pmm", bufs=2)

                def stat(pair_c0, cross_c):
                    if pair_c0 is not None:  # hi*hi: slots (kc0,hi),(kc1,hi)
                        return xw[:, pair_c0 : pair_c0 + 2, 0, tr]
                    return xw[:, cross_c, :, tr]  # (hi,lo)

                def mov(pair_c0, cross_c):
                    if pair_c0 is not None:  # slots (kc0,hi),(kc1,hi)
                        return w8_r[:, pair_c0 : pair_c0 + 2, 1, 512:768]
                    return w8_r[:, cross_c, :, 512:768]  # (lo,hi)

                mm_pairs(pp[:, 0:256], stat, mov)
                vt = v_aug[:, tt * HPC * 65 : (tt + 1) * HPC * 65].rearrange(
                    "p (h c) -> p h c", c=65
                )[:, :, 0:64]
                nc.vector.scalar_tensor_tensor(
                    out=vt,
                    in0=pp[:, 0:256].rearrange("p (h c) -> p h c", c=64),
                    scalar=QKV_DESCALE,
                    in1=vb_bc[:].rearrange("p (h c) -> p h c", c=64),
                    op0=mybir.AluOpType.mult,
                    op1=mybir.AluOpType.add,
                )

            def emit_qk(i, tch):
                PHASE_MARKS.append((f"QK({i},{tch})", len(nc.inst_map)))
                xw = x8_w(tch)
                ir = slice(i * 128, (i + 1) * 128)
                pp0 = ps.tile([128, 512], F32, tag="pmm", bufs=2)
                pp = pp0[:]

                def stat(pair_c0, cross_c):
                    if pair_c0 is not None:
                        return w8_r[:, pair_c0 : pair_c0 + 2, 1, ir]
                    return w8_r[:, cross_c, :, ir]

                def mov(pair_c0, cross_c):
                    if pair_c0 is not None:
                        return xw[:, pair_c0 : pair_c0 + 2, 0, :]
                    return xw[:, cross_c, :, :]

                mm_pairs(pp, stat, mov)
                dst = qkT[:, i * T + tch * 512 : i * T + (tch + 1) * 512]
                # DVE, not ACT: evacuations on ACT would queue ahead of the
                # attention exps and stretch the S-tile free latency
                nc.vector.tensor_scalar(
                    out=dst, in0=pp, scalar1=QKV_DESCALE,
                    scalar2=qkb[:, i : i + 1],
                    op0=mybir.AluOpType.mult, op1=mybir.AluOpType.add,
                )

            def emit_att_head(sc, h, fillers=None, make_fp8=True):
                PHASE_MARKS.append((f"att{sc}h{h}", len(nc.inst_map)))
                qT = qkT[64 * (h % 2) : 64 * (h % 2) + 64, (h // 2) * T : (h // 2 + 1) * T]
                kT = qkT[64 * (h % 2) : 64 * (h % 2) + 64, (2 + h // 2) * T : (3 + h // 2) * T]
                oT_ps = ps.tile([65, 512], F32, tag="ot", bufs=2)
                n_kj = (sc + 1) * 4
                npairs = n_kj // 2

                def pair_layout(p):
                    # [(bank_off, q_off, cols, kj), ...]; pack both k tiles
                    # into one PSUM bank when their columns fit (saves exp
                    # span and a bank)
                    kj0, kj1 = 2 * p, 2 * p + 1
                    qo0 = max(0, kj0 * 128 - sc * 512)
                    qo1 = max(0, kj1 * 128 - sc * 512)
                    c0, c1 = 512 - qo0, 512 - qo1
                    if c0 + c1 <= 512:
                        return [(0, qo0, c0, kj0), (c0, qo1, c1, kj1)], c0 + c1
                    return [(0, qo0, c0, kj0), (512, qo1, c1, kj1)], 512 + c1

                def emit_s_pair(p):
                    layout, span = pair_layout(p)
                    one_bank = layout[1][0] < 512
                    st = ps.tile([128, 1024], F32, tag="smm", bufs=2)
                    pt = stage.tile([128, 1024], F16, tag="pt", bufs=4)
                    # per-PSUM-bank balanced start/stop: in the one_bank pack
                    # both k-tiles share a group (start zeroes the whole zero
                    # region, so the second k-tile's disjoint columns
                    # accumulate onto zeros); otherwise one group per bank
                    for idx, (boff, q_off, cols, kj) in enumerate(layout):
                        diag = kj >= sc * 4
                        first = idx == 0 or not one_bank
                        last_in_group = (not one_bank) or idx == 1
                        nc.tensor.matmul(
                            st[:, boff : boff + cols],
                            kT[:, kj * 128 : (kj + 1) * 128],
                            qT[:, sc * 512 + q_off : (sc + 1) * 512],
                            start=first,
                            stop=(not diag) and last_in_group,
                        )
                        if diag:
                            nc.tensor.matmul(
                                st[:, boff : boff + 128], mstat[:], mmov[:],
                                start=False, stop=last_in_group,
                            )
                    # one exp covering both halves (cols between valid ranges
                    # hold stale PSUM; the pt garbage there is never read)
                    nc.scalar.activation(
                        pt[:, :span], st[:, :span],
                        mybir.ActivationFunctionType.Exp,
                        scale=EXP_SCALE,
                    )
                    return pt, layout

                def emit_pv_pair(p, pt, layout):
                    for boff, q_off, cols, kj in layout:
                        vv = v_aug[:, (kj * HPC + h) * 65 : (kj * HPC + h + 1) * 65]
                        nc.tensor.matmul(
                            oT_ps[:, q_off:512],
                            vv,
                            pt[:, boff : boff + cols],
                            start=(kj == 0),
                            stop=(kj == n_kj - 1),
                        )

                # software pipeline: emit S(p+1) before PV(p) so PE always has
                # matmul work queued while exp(p) completes on ACT; fillers
                # (independent work units, one LIST per exp-wait slot) absorb
                # ACT-bound gaps; leftovers flush before the final PV so they
                # still precede the normalize chain
                fillers = [list(f) for f in (fillers or [])]
                pend = emit_s_pair(0)
                for p in range(1, npairs):
                    nxt = emit_s_pair(p)
                    if fillers:
                        for f in fillers.pop(0):
                            f()
                    emit_pv_pair(p - 1, *pend)
                    pend = nxt
                for fl in fillers:
                    for f in fl:
                        f()
                emit_pv_pair(npairs - 1, *pend)
                # normalize chain for this head (DVE + Pool), frees oT_ps;
                # rowsum scaled by 1/OS so oT carries a xOS pre-scale
                rs = stage.tile([1, 512], F32, tag="rs", bufs=4)
                recip = stage.tile([1, 512], F32, tag="recip", bufs=4)
                bc = stage.tile([64, 512], F32, tag="bc", bufs=4)
                if sc == 2 and h == 1:
                    # last head: pipeline the rowsum->recip->broadcast chain
                    # in column halves across ACT/DVE/Pool so the first fin
                    # closes unblock ~0.5us earlier
                    for hf in range(2):
                        hs = slice(hf * 256, (hf + 1) * 256)
                        nc.scalar.activation(
                            rs[:, hs], oT_ps[64:65, hs],
                            mybir.ActivationFunctionType.Copy, scale=1.0 / OS,
                        )
                        nc.vector.reciprocal_approx_fast(recip[:, hs], rs[:, hs])
                        nc.gpsimd.partition_broadcast(bc[:, hs], recip[:, hs])
                else:
                    nc.vector.tensor_scalar_mul(rs[:], oT_ps[64:65, :], 1.0 / OS)
                    nc.vector.reciprocal_approx_fast(recip[:], rs[:])
                    nc.gpsimd.partition_broadcast(bc[:], recip[:])
                o16 = oT_sb[sc % 2][
                    64 * (h % 2) : 64 * (h % 2) + 64,
                    (h // 2) * 512 : (h // 2 + 1) * 512,
                ]
                if sc == 2 and h == 1:
                    # last head: 128-col pieces so each fin close piece j
                    # unblocks as soon as its oT columns land
                    for q4 in range(4):
                        qs = slice(q4 * 128, (q4 + 1) * 128)
                        nc.vector.tensor_mul(
                            o16[:, qs], oT_ps[0:64, qs], bc[:, qs]
                        )
                else:
                    nc.vector.tensor_mul(o16, oT_ps[0:64, :], bc[:])
                if make_fp8:
                    # both on Pool: keeps the DVE queue clear for the next
                    # head's rs/recip/mul chain (Pool has slack and its
                    # in-queue delay hides under the next head's PV)
                    pr = 64 * (h % 2)
                    hi = oT8_r[sc % 2][pr : pr + 64, h // 2, 0, :]
                    lo = oT8_r[sc % 2][pr : pr + 64, h // 2, 1, :]
                    nc.gpsimd.tensor_copy(hi, o16)
                    nc.gpsimd.tensor_sub(lo, o16, hi)

            def emit_proj_piece(sc, j, evac_on_act=None, dmaq=None):
                # fp8 path: project rows [sc*512 + j*128, +128).
                # DVE evac: stt folds descale + bias. ACT evac: bias enters
                # PSUM via a K=1 ones x (bias/descale) matmul, then a scaled
                # Copy (Pool cannot read PSUM).
                PHASE_MARKS.append((f"proj({sc},{j})", len(nc.inst_map)))
                r0 = sc * 512 + j * 128
                o8 = oT8_r[sc % 2]
                jr = slice(j * 128, (j + 1) * 128)
                on_act = bool(evac_on_act)
                pst = stage.tile([128, 1024], F16, tag="pst", bufs=4)
                for nch in range(2):
                    nr = slice(nch * 512, (nch + 1) * 512)
                    pp = ps.tile([128, 512], F32, tag="pmm", bufs=2)
                    nc.tensor.matmul(
                        pp[:],
                        o8[:, :, 0, jr],        # (pair0 hi, pair1 hi)
                        wpa8_r[:, :, 1, nr],    # (pair0 hi, pair1 hi)
                        start=True, stop=False, perf_mode=DR,
                    )
                    if on_act:
                        nc.tensor.matmul(
                            pp[:], ones_col[:],
                            bpr[:, C + nch * 512 : C + (nch + 1) * 512],
                            start=False, stop=False,
                        )
                    for p in range(2):
                        nc.tensor.matmul(
                            pp[:],
                            o8[:, p, :, jr],      # (hi, lo)
                            wpa8_r[:, p, :, nr],  # (lo, hi)
                            start=False, stop=(p == 1), perf_mode=DR,
                        )
                    if on_act:
                        nc.scalar.activation(
                            pst[:, nr], pp[:], mybir.ActivationFunctionType.Copy,
                            scale=PROJ8_DESCALE,
                        )
                    else:
                        nc.vector.scalar_tensor_tensor(
                            out=pst[:, nr], in0=pp[:], scalar=PROJ8_DESCALE,
                            in1=bias_bc[:, nr],
                            op0=mybir.AluOpType.mult, op1=mybir.AluOpType.add,
                        )
                # spread partial-write issues across SEQ queues: each DMA
                # issue occupies its queue ~650ns and the tail needs several
                # in flight at once
                dq = dmaq if dmaq is not None else [nc.sync, nc.scalar][j % 2]
                dq.dma_start(partial_d[r0 : r0 + 128, :], pst[:])

            def emit_rs(gi):
                r0, r1 = RS_GROUPS[gi]
                nc.gpsimd.collective_compute(
                    "ReduceScatter",
                    mybir.AluOpType.add,
                    replica_groups=[[0, 1, 2, 3], [4, 5, 6, 7]],
                    ins=[partial_d[r0:r1, :]],
                    outs=[rsout_d[gi][:]],
                )

            def emit_out_copy(gi):
                # deferred to the tail: an out-copy waiting on its collective
                # must not sit in an in-order DMA queue ahead of partial
                # writes that later collectives depend on
                r0, r1 = RS_GROUPS[gi]
                og = sum((b1 - b0) // 4 for (b0, b1) in RS_GROUPS[:gi])
                ln4 = (r1 - r0) // 4
                # huge scheduling-floor: keeps the tile scheduler from
                # hoisting these RS-gated copies ahead of the tail partial
                # writes in the same queue (ordering hint only, no HW wait)
                with tc.tile_wait_until(1.0):
                    nc.sync.dma_start(out_part[og : og + ln4, :], rsout_d[gi][:])

            # ---------------- schedule ----------------
            # PE warmup: the tensor engine clock ramps with sustained use and
            # the first ~9us are DMA-bound anyway, so spin dependency-free
            # dummy matmuls to hit full p-state before real work arrives.
            warm = ps.tile([128, 1024], F32, tag="smm", bufs=2)
            for i in range(120):
                nc.tensor.matmul(
                    warm[:, (i % 8) * 128 : (i % 8 + 1) * 128],
                    mstat[:], mmov[:],
                    start=True, stop=True, skip_group_check=True,
                )

            # Sub-chunk order: sc0, sc1, sc3, sc2. The LAST chunk's
            # projection gates the final ReduceScatter, so the smallest
            # remaining triangle (sc2) goes last; fin2 is the f16 tail path.
            # fp8 matmuls outrun the input DMAs at the start, so only the
            # QK(tch0) units precede att0 and everything else fills
            # attention's exp-wait slots in DMA-arrival order.
            for i in (1, 3, 0, 2):
                emit_qk(i, 0)

            # att sc0; V(0..3) land between S and PV; everything att1
            # needs fills the rest of att0's exp-wait slots
            att0_fill = [
                [[lambda: emit_v(0), lambda: emit_v(1)],
                 [lambda: emit_v(2), lambda: emit_v(3)]],
                [[lambda: emit_v(4)], [lambda: emit_v(5)]],
                [[lambda: emit_v(6)], [lambda: emit_qk(1, 1)]],
                [[lambda: emit_v(7)], [lambda: emit_qk(3, 1)]],
            ]
            for k, h in enumerate((2, 3, 0, 1)):
                emit_att_head(0, h, att0_fill[k])

            # att sc1; x-window-2/3 units + sc0 proj (oT8[0] is stable from
            # here on) fill the slots
            att1_fill = [
                [[lambda: emit_qk(0, 1)], [lambda: emit_v(8)],
                 [lambda: emit_v(9)]],
                [[lambda: emit_qk(2, 1)], [lambda: emit_v(10)],
                 [lambda: emit_v(11)]],
                [[lambda: emit_proj_piece(0, 0)], [lambda: emit_v(12)],
                 [lambda: emit_v(13)]],
                [[lambda: emit_proj_piece(0, 1)], [lambda: emit_v(14)],
                 [lambda: emit_v(15)]],
            ]
            for k, h in enumerate((2, 3, 0, 1)):
                emit_att_head(1, h, att1_fill[k])

            # fin1: att3's own Q/K units lead (covering the last sc1 head's
            # normalize chain), then the sc1 projection (it reads oT8[1],
            # which att3 overwrites -- the WAR dep keeps the reads safe)
            emit_qk(1, 3)
            emit_qk(3, 3)
            emit_qk(3, 2)
            emit_proj_piece(1, 0)
            emit_proj_piece(1, 1)
            emit_proj_piece(1, 2)
            emit_proj_piece(1, 3)

            # att sc3 (biggest triangle): remaining K/Q units for its own
            # later heads + sc0's remaining proj (oT8[0] stays untouched)
            att3_fill = [
                [[lambda: emit_qk(2, 2)], [lambda: emit_qk(2, 3)],
                 [lambda: emit_qk(0, 3)]],
                [[lambda: emit_qk(0, 2)], [lambda: emit_proj_piece(0, 2)]],
                [[lambda: emit_proj_piece(0, 3)]],
                [[lambda: emit_rs(0)]],
            ]
            for k, h in enumerate((2, 3, 0, 1)):
                emit_att_head(3, h, att3_fill[k])
            # fin3 lead-in: one ready QK unit covers the last sc3 head's
            # chain, then the first sc3 proj pieces
            emit_qk(1, 2)
            emit_proj_piece(3, 0)
            emit_proj_piece(3, 1)

            # att sc2 (last): rest of fin3 + its rs early; sc2 skips the fp8
            # split (f16 tail path).
            att2_fill = {
                2: [[lambda: emit_proj_piece(3, 2)]],
                3: [[lambda: emit_proj_piece(3, 3)]],
                0: [[lambda: emit_rs(1)]],
            }
            for k, h in enumerate((2, 3, 0, 1)):
                emit_att_head(2, h, att2_fill.get(h), make_fp8=False)

            # fin2: f16 tail projection for sc2 (no bias matmuls on the DVE
            # pieces; bias + 1/OS descale folded into the stt evacuation)
            oT_cur = oT_sb[0]
            tail_q = [nc.gpsimd, nc.scalar, nc.sync, nc.sync]
            tail_act = [True, True, False, False]

            def fin_open(j, kind):
                # kind: "smm" -> one [128,1024] tile split in halves;
                # "pmm" -> two [128,512] tiles (attention is done, both
                # pools are free -- this lets 3 opens stay in flight)
                PHASE_MARKS.append((f"fin_open{j}", len(nc.inst_map)))
                if kind == "smm":
                    t = ps.tile([128, 1024], F32, tag="smm", bufs=2)
                    aps = [t[:, 0:512], t[:, 512:1024]]
                else:
                    aps = []
                    for _n in range(2):
                        fpp = ps.tile(
                            [128, 512], F32, tag="pmm", bufs=2, name=f"fpp{_n}"
                        )
                        aps.append(fpp[:])
                for nch in range(2):
                    nc.tensor.matmul(
                        aps[nch],
                        oT_cur[:, 512 + j * 128 : 512 + (j + 1) * 128],
                        wpa16[:, C + nch * 512 : C + (nch + 1) * 512],
                        start=True, stop=False,
                    )
                    if tail_act[j]:
                        nc.tensor.matmul(
                            aps[nch],
                            ones_col[:],
                            bpr[:, 2 * C + nch * 512 : 2 * C + (nch + 1) * 512],
                            start=False, stop=False,
                        )
                return aps

            def fin_close(j, aps):
                PHASE_MARKS.append((f"fin_close{j}", len(nc.inst_map)))
                for nch in range(2):
                    nc.tensor.matmul(
                        aps[nch],
                        oT_cur[:, j * 128 : (j + 1) * 128],
                        wpa16[:, nch * 512 : (nch + 1) * 512],
                        start=False, stop=True,
                    )
                pst = stage.tile([128, 1024], F16, tag="pst", bufs=4)
                for nch in range(2):
                    nr = slice(nch * 512, (nch + 1) * 512)
                    if tail_act[j]:
                        nc.scalar.activation(
                            pst[:, nr], aps[nch],
                            mybir.ActivationFunctionType.Copy,
                            scale=PROJ16_DESCALE,
                        )
                    else:
                        nc.vector.scalar_tensor_tensor(
                            out=pst[:, nr], in0=aps[nch], scalar=PROJ16_DESCALE,
                            in1=bias_bc[:, nr],
                            op0=mybir.AluOpType.mult, op1=mybir.AluOpType.add,
                        )
                r0 = 2 * 512 + j * 128
                tail_q[j].dma_start(partial_d[r0 : r0 + 128, :], pst[:])

            if DEBUG:
                nc.sync.dma_start(dbg["oT1"][:], oT_sb[1][:])
                nc.sync.dma_start(dbg["oT81"][:], oT8_sb[1][:])
                nc.sync.dma_start(dbg["qkT"][:], qkT[:])
            pps = [fin_open(0, "smm"), fin_open(1, "smm"), fin_open(2, "pmm")]
            fin_close(0, pps[0])
            pp3 = fin_open(3, "smm")
            fin_close(1, pps[1])
            fin_close(2, pps[2])
            fin_close(3, pp3)
            emit_rs(2)
            for gi in range(len(RS_GROUPS)):
                emit_out_copy(gi)
            if DEBUG:
                nc.sync.dma_start(dbg["partial"][:], partial_d[:])

    nc.finalize()
    return nc


_NC = None


def _get_nc():
    global _NC
    if _NC is None:
        _NC = _build()
    return _NC


def _perm_qkv(w):
    # (..., h*192 + t*64 + c) -> (..., t*256 + h*64 + c)
    s = w.shape[:-1]
    return np.ascontiguousarray(
        w.reshape(*s, HPC, 3, HD).swapaxes(-3, -2).reshape(*s, CG)
    )


def _hilo(a):
    hi = a.astype(E4)
    lo = (a - hi.astype(np.float32)).astype(E4)
    return hi, lo


def _make_in_maps(x, Wqkv, bqkv, Wproj, bproj):
    x = np.asarray(x, dtype=np.float32)
    Wqkv = np.asarray(Wqkv, dtype=np.float32)
    bqkv = np.asarray(bqkv, dtype=np.float32)
    Wproj = np.asarray(Wproj, dtype=np.float32)
    bproj = np.asarray(bproj, dtype=np.float32)

    in_maps = []
    for c in range(N_CORES):
        b, g = divmod(c, 4)
        # x8: [p, w, kc, e(hi,lo), t]
        xT = x[b].T * XS  # (C, T)
        xa = xT.reshape(KC, 128, 4, 512).transpose(1, 2, 0, 3)  # [p, w, kc, t]
        xh, xl = _hilo(xa)
        x8 = np.stack([xh, xl], axis=3).reshape(128, -1)
        # w8: [p, kc, e(lo,hi), m]
        wp_ = _perm_qkv(Wqkv[:, g * CG : (g + 1) * CG]) * WS
        wa = wp_.reshape(KC, 128, CG).transpose(1, 0, 2)  # [p, kc, m]
        wh, wl = _hilo(wa)
        w8 = np.stack([wl, wh], axis=2).reshape(128, -1)
        bq = _perm_qkv(bqkv[g * CG : (g + 1) * CG])
        qkb = np.ascontiguousarray(bq[:512].reshape(4, 128).T).astype(np.float32)
        vb = bq[512:768].reshape(1, 256).astype(np.float16)
        # wpa: [p, pair, c]
        wpa = np.zeros((128, 2, C), np.float32)
        for pair in range(2):
            wpa[:, pair] = Wproj[
                g * 256 + pair * 128 : g * 256 + (pair + 1) * 128, :
            ]
        ph, pl = _hilo(wpa * WS)
        wpa8 = np.stack([pl, ph], axis=2).reshape(128, -1)
        b0 = bproj if g == 0 else np.zeros(C, np.float32)
        bpr = np.concatenate([b0, b0 * OS * WS, b0 * OS]).reshape(1, 3 * C)
        in_maps.append(
            {
                "x8": x8,
                "w8": w8,
                "qkb": qkb,
                "vb": vb,
                "wpa8": wpa8,
                "wpa16": wpa.reshape(128, -1).astype(np.float16),
                "bpr": bpr.astype(np.float16),
            }
        )
    return in_maps


def _run(in_maps, trace=False):
    nc = _get_nc()
    return run_bass_kernel_spmd(nc, in_maps, list(range(N_CORES)), trace=trace)


def kernel(x, Wqkv, bqkv, Wproj, bproj):
    in_maps = _make_in_maps(x, Wqkv, bqkv, Wproj, bproj)
    res = _run(in_maps)
    out = np.empty((B, T, C), np.float32)
    for c in range(N_CORES):
        b, g = divmod(c, 4)
        op = res.results[c]["out_part"].astype(np.float32)
        og = 0
        for r0, r1 in RS_GROUPS:
            ln4 = (r1 - r0) // 4
            out[b, r0 + g * ln4 : r0 + (g + 1) * ln4, :] = op[og : og + ln4]
            og += ln4
    return out


# revision 77
# speedup vs baseline: 1.0020x; 1.0020x over previous
"""Multi-head causal self-attention (B=2, T=2048, C=1024, H=16) on 8 trn2 cores.

Sharding: data-parallel over batch (2) x tensor-parallel over heads (4 groups
of 4 heads). Core c handles batch b=c//4, head group g=c%4.

Key structure (per core):
  - QKV and output projections run in fp8e4m3 DoubleRow with hi+lo error
    compensation: every operand X is host- (or device-) split into
    X_hi = fp8(X), X_lo = fp8(X - X_hi); each 256-channel contraction uses
    3 DoubleRow matmuls (hi*hi packing two k-tiles, plus one cross-term
    matmul per k-tile computing hi*lo + lo*hi in its two slots) instead of
    2 f16 matmuls -> 4x per-row speed at 3/2 the instruction count = 2.67x,
    with quantization error compensated to ~1e-3. Operands are pre-scaled
    (x*8, W*64, oT*16) to clear the e4m3 subnormal floor; descale happens
    in the PSUM evacuation op.
  - Attention stays f16 in S^T orientation (k on partitions, q free): the
    exp on ACT (~58us) is the attention-phase floor, so cheaper PE matmuls
    there would buy nothing. Causal mask via a PE add-matmul (-200
    strict-upper-tri stationary x identity moving).
  - Rowsums from a ones-column appended to V; recip on DVE (scaled 1/16 so
    oT carries a x16 pre-scale for its fp8 split); partition_broadcast on
    Pool.
  - All projection bias adds are folded into the PSUM evacuation
    (scalar_tensor_tensor with descale scalar + partition-broadcast bias)
    on DVE/Pool -- no bias matmuls on PE.
  - Sub-chunk processing order is sc0, sc1, sc3, sc2: the final
    ReduceScatter is gated by the last chunk's projection, so the smallest
    remaining causal triangle (sc2) goes last; its projection stays f16
    (reads oT directly, skipping the fp8 hi/lo split latency) to keep the
    tail chain short. PSUM is split into 2x[128,1024] S-tile buffers and
    2x[128,512] buffers for QKV/proj units so fillers never contend with
    the attention pipeline; PE warms its p-state on dummy matmuls during
    the DMA-bound preamble.
"""

import os

import numpy as np
import ml_dtypes

import concourse.bacc as bacc
import concourse.mybir as mybir
import concourse.tile as tile
from concourse.bass_utils import run_bass_kernel_spmd

DEBUG = bool(int(os.environ.get("KERNEL_DEBUG", "0")))
PHASE_MARKS = []  # (label, first_instruction_index) for trace attribution

F32 = mybir.dt.float32
F16 = mybir.dt.float16
F8 = mybir.dt.float8e4
E4 = ml_dtypes.float8_e4m3
DR = mybir.MatmulPerfMode.DoubleRow

B, T, C, H = 2, 2048, 1024, 16
HPC = 4                 # heads per core
HD = 64                 # head dim
CG = HPC * 3 * HD       # 768 qkv cols per core
KC = 8                  # contraction chunks (128 channels each)
TT = T // 128           # 16 k tiles
NSC = T // 512          # 4 q sub-chunks
N_CORES = 8
EXP_SCALE = 0.125

XS = 8.0                # host pre-scale on x
WS = 64.0               # host pre-scale on Wqkv / Wproj
OS = 16.0               # device pre-scale on attention output (via recip)
QKV_DESCALE = 1.0 / (XS * WS)
PROJ8_DESCALE = 1.0 / (OS * WS)
PROJ16_DESCALE = 1.0 / OS

# reduce-scatter groups as (row_start, row_end) in COMPLETION order
# (sc0+sc1, then sc3, then sc2 last); each core keeps len/4 rows
RS_GROUPS = [(0, 1024), (1536, 2048), (1024, 1536)]


def _build():
    nc = bacc.Bacc(None, target_bir_lowering=False)

    # x8: [p, w(4), kc(8), e(hi,lo), t(512)] fp8
    x8_in = nc.dram_tensor("x8", [128, 4 * KC * 2 * 512], F8, kind="ExternalInput")
    # w8: [p, kc(8), e(lo,hi), m(768)] fp8
    w8_in = nc.dram_tensor("w8", [128, KC * 2 * CG], F8, kind="ExternalInput")
    qkb_in = nc.dram_tensor("qkb", [128, 4], F32, kind="ExternalInput")
    vb_in = nc.dram_tensor("vb", [1, 256], F16, kind="ExternalInput")
    # wpa8: [p, pair(2), e(lo,hi), c(1024)] fp8 (x64 scaled Wproj rows)
    wpa8_in = nc.dram_tensor("wpa8", [128, 2 * 2 * C], F8, kind="ExternalInput")
    # wpa16: [p, pair(2)*c] f16 unscaled Wproj rows (tail path)
    wpa16_in = nc.dram_tensor("wpa16", [128, 2 * C], F16, kind="ExternalInput")
    # bpr blocks along free dim: [bproj | bproj*OS*WS | bproj*OS]
    # (blocks 1,2 are added in PSUM via a K=1 matmul before a scaled ACT evac)
    bpr_in = nc.dram_tensor("bpr", [1, 3 * C], F16, kind="ExternalInput")
    out_part = nc.dram_tensor("out_part", [T // 4, C], F16, kind="ExternalOutput")

    partial_d = nc.dram_tensor("partial_d", [T, C], F16)
    dbg = {}
    if DEBUG:
        dbg["oT1"] = nc.dram_tensor("dbg_oT1", [128, 2 * 512], F16, kind="ExternalOutput")
        dbg["oT81"] = nc.dram_tensor("dbg_oT81", [128, 4 * 512], F8, kind="ExternalOutput")
        dbg["partial"] = nc.dram_tensor("dbg_partial", [T, C], F16, kind="ExternalOutput")
        dbg["qkT"] = nc.dram_tensor("dbg_qkT", [128, 4 * T], F16, kind="ExternalOutput")
    rsout_d = [
        nc.dram_tensor(f"rsout_d{i}", [(r1 - r0) // 4, C], F16)
        for i, (r0, r1) in enumerate(RS_GROUPS)
    ]

    with tile.TileContext(nc) as tc:
        with (
            tc.tile_pool(name="cpool", bufs=1) as cpool,
            tc.tile_pool(name="main", bufs=1) as main,
            tc.tile_pool(name="stage", bufs=1) as stage,
            tc.tile_pool(name="ps", bufs=1, space="PSUM") as ps,
        ):
            # ---------------- constants ----------------
            vb_bc = cpool.tile([128, 256], F16)
            bias_bc = cpool.tile([128, C], F16)
            ones_col = cpool.tile([1, 128], F16)
            nc.vector.memset(ones_col[:], 1.0)
            # mask stationary: mstat[f, p] = -200 where p > f else 0
            mstat = cpool.tile([128, 128], F16)
            nc.gpsimd.memset(mstat[:], -200.0)
            nc.gpsimd.affine_select(
                out=mstat[:], in_=mstat[:],
                compare_op=mybir.AluOpType.is_ge, fill=0.0,
                base=-1, pattern=[[1, 128]], channel_multiplier=-1,
            )
            # mask moving: identity
            mmov = cpool.tile([128, 128], F16)
            nc.gpsimd.memset(mmov[:], 0.0)
            nc.gpsimd.affine_select(
                out=mmov[:], in_=mmov[:],
                compare_op=mybir.AluOpType.not_equal, fill=1.0,
                base=0, pattern=[[-1, 128]], channel_multiplier=1,
            )

            # ---------------- persistent tensors ----------------
            x8 = main.tile([128, 4 * KC * 2 * 512], F8)
            w8 = main.tile([128, KC * 2 * CG], F8)
            qkb = main.tile([128, 4], F32)
            vb = main.tile([1, 256], F16)
            wpa8 = main.tile([128, 2 * 2 * C], F8)
            wpa16 = main.tile([128, 2 * C], F16)
            bpr = main.tile([1, 3 * C], F16)
            qkT = main.tile([128, 4 * T], F16)             # [Q01;Q23;K01;K23] x T
            v_aug = main.tile([128, TT * HPC * 65], F16)   # per (tt,h): 64 V + ones col
            # f16 attention outs, [pair][512] col layout (x16 pre-scaled);
            # head h lives at partitions 64*(h%2).., column block (h//2)*512
            oT_sb = [
                main.tile([128, 2 * 512], F16, name=f"oT_sb{i}") for i in range(2)
            ]
            # fp8 hi/lo split of oT for the fp8 projection path:
            # [p, pair(2), e(hi,lo), 512]
            oT8_sb = [
                main.tile([128, 2 * 2 * 512], F8, name=f"oT8_sb{i}") for i in range(2)
            ]

            nc.vector.memset(v_aug[:], 1.0)  # ones columns give softmax rowsums

            # ---------------- views ----------------
            w8_r = w8[:].rearrange("p (kc e m) -> p kc e m", kc=KC, e=2)
            w8_in_r = w8_in[:].rearrange("p (kc e m) -> p kc e m", kc=KC, e=2)
            wpa8_r = wpa8[:].rearrange("p (pr e c) -> p pr e c", pr=2, e=2)
            oT8_r = [
                t[:].rearrange("p (pr e c) -> p pr e c", pr=2, e=2) for t in oT8_sb
            ]

            def x8_w(w):
                # [128, kc, e, 512] view of window w
                return x8[:, w * 8192 : (w + 1) * 8192].rearrange(
                    "p (kc e t) -> p kc e t", kc=KC, e=2
                )

            # ---------------- input DMAs ----------------
            # DMA transfers serialize on the DMA-engine pool, so order them
            # by what gates compute: x window 0 + QK weights interleaved per
            # kc pair (attention sc0's S matmuls are the longest dependency
            # chain), V weights next (PV trails S by ~1.5us), then the later
            # x windows.
            for kp in range(4):
                nc.sync.dma_start(
                    x8[:, kp * 2048 : (kp + 1) * 2048],
                    x8_in[:, kp * 2048 : (kp + 1) * 2048],
                )
                nc.sync.dma_start(
                    w8_r[:, 2 * kp : 2 * kp + 2, :, 0:512],
                    w8_in_r[:, 2 * kp : 2 * kp + 2, :, 0:512],
                )
            nc.sync.dma_start(vb[:], vb_in[:])
            nc.sync.dma_start(qkb[:], qkb_in[:])
            nc.gpsimd.partition_broadcast(vb_bc[:], vb[:])
            nc.sync.dma_start(w8_r[:, 0:4, :, 512:768], w8_in_r[:, 0:4, :, 512:768])
            nc.sync.dma_start(w8_r[:, 4:8, :, 512:768], w8_in_r[:, 4:8, :, 512:768])
            nc.sync.dma_start(x8[:, 8192:16384], x8_in[:, 8192:16384])      # w1
            nc.scalar.dma_start(wpa8[:], wpa8_in[:])
            nc.scalar.dma_start(bpr[:], bpr_in[:])
            nc.gpsimd.partition_broadcast(bias_bc[:], bpr[:, 0:C])
            nc.sync.dma_start(x8[:, 16384:24576], x8_in[:, 16384:24576])    # w2
            nc.sync.dma_start(x8[:, 24576:32768], x8_in[:, 24576:32768])    # w3
            nc.scalar.dma_start(wpa16[:], wpa16_in[:])

            # ---------------- emit helpers ----------------
            def mm_pairs(pp_slice, stat_of, mov_of):
                """Emit the compensated fp8 matmul group: for each kc pair,
                hi*hi (2 slots = the 2 k-tiles), then a cross matmul per
                k-tile (slots = hi*lo + lo*hi)."""
                n = KC // 2
                for p in range(n):
                    c0 = 2 * p
                    nc.tensor.matmul(
                        pp_slice, stat_of(c0, None), mov_of(c0, None),
                        start=(p == 0), stop=False, perf_mode=DR,
                    )
                    for c in (c0, c0 + 1):
                        nc.tensor.matmul(
                            pp_slice, stat_of(None, c), mov_of(None, c),
                            start=False, stop=(c == KC - 1), perf_mode=DR,
                        )

            def emit_v(tt):
                PHASE_MARKS.append((f"V{tt}", len(nc.inst_map)))
                w, tloc = divmod(tt, 4)
                xw = x8_w(w)
                tr = slice(tloc * 128, (tloc + 1) * 128)
                pp = ps.tile([128, 512], F32, tag="pmm", bufs=2)

                def stat(pair_c0, cross_c):
                    if pair_c0 is not None:  # hi*hi: slots (kc0,hi),(kc1,hi)
                        return xw[:, pair_c0 : pair_c0 + 2, 0, tr]
                    return xw[:, cross_c, :, tr]  # (hi,lo)

                def mov(pair_c0, cross_c):
                    if pair_c0 is not None:  # slots (kc0,hi),(kc1,hi)
                        return w8_r[:, pair_c0 : pair_c0 + 2, 1, 512:768]
                    return w8_r[:, cross_c, :, 512:768]  # (lo,hi)

                mm_pairs(pp[:, 0:256], stat, mov)
                vt = v_aug[:, tt * HPC * 65 : (tt + 1) * HPC * 65].rearrange(
                    "p (h c) -> p h c", c=65
                )[:, :, 0:64]
                nc.vector.scalar_tensor_tensor(
                    out=vt,
                    in0=pp[:, 0:256].rearrange("p (h c) -> p h c", c=64),
                    scalar=QKV_DESCALE,
                    in1=vb_bc[:].rearrange("p (h c) -> p h c", c=64),
                    op0=mybir.AluOpType.mult,
                    op1=mybir.AluOpType.add,
                )

            def emit_qk(i, tch):
                PHASE_MARKS.append((f"QK({i},{tch})", len(nc.inst_map)))
                xw = x8_w(tch)
                ir = slice(i * 128, (i + 1) * 128)
                pp0 = ps.tile([128, 512], F32, tag="pmm", bufs=2)
                pp = pp0[:]

                def stat(pair_c0, cross_c):
                    if pair_c0 is not None:
                        return w8_r[:, pair_c0 : pair_c0 + 2, 1, ir]
                    return w8_r[:, cross_c, :, ir]

                def mov(pair_c0, cross_c):
                    if pair_c0 is not None:
                        return xw[:, pair_c0 : pair_c0 + 2, 0, :]
                    return xw[:, cross_c, :, :]

                mm_pairs(pp, stat, mov)
                dst = qkT[:, i * T + tch * 512 : i * T + (tch + 1) * 512]
                # DVE, not ACT: evacuations on ACT would queue ahead of the
                # attention exps and stretch the S-tile free latency
                nc.vector.tensor_scalar(
                    out=dst, in0=pp, scalar1=QKV_DESCALE,
                    scalar2=qkb[:, i : i + 1],
                    op0=mybir.AluOpType.mult, op1=mybir.AluOpType.add,
                )

            def emit_qk2(ia, ib, tch):
                # two QK units interleaved by kc pair: during the DMA-bound
                # preamble neither unit serializes behind the other's stall,
                # so both evacuate as soon as the last kc pieces land
                PHASE_MARKS.append((f"QK2({ia},{ib},{tch})", len(nc.inst_map)))
                xw = x8_w(tch)
                pps = {}
                for i in (ia, ib):
                    pps[i] = ps.tile(
                        [128, 512], F32, tag="pmm", bufs=2, name=f"qk2_{i}"
                    )
                for p in range(KC // 2):
                    c0 = 2 * p
                    for i in (ia, ib):
                        ir = slice(i * 128, (i + 1) * 128)
                        nc.tensor.matmul(
                            pps[i][:], w8_r[:, c0 : c0 + 2, 1, ir],
                            xw[:, c0 : c0 + 2, 0, :],
                            start=(p == 0), stop=False, perf_mode=DR,
                        )
                        for c in (c0, c0 + 1):
                            nc.tensor.matmul(
                                pps[i][:], w8_r[:, c, :, ir], xw[:, c, :, :],
                                start=False, stop=(c == KC - 1), perf_mode=DR,
                            )
                for i in (ia, ib):
                    dst = qkT[:, i * T + tch * 512 : i * T + (tch + 1) * 512]
                    nc.vector.tensor_scalar(
                        out=dst, in0=pps[i][:], scalar1=QKV_DESCALE,
                        scalar2=qkb[:, i : i + 1],
                        op0=mybir.AluOpType.mult, op1=mybir.AluOpType.add,
                    )

            def emit_att_head(sc, h, fillers=None, make_fp8=True):
                PHASE_MARKS.append((f"att{sc}h{h}", len(nc.inst_map)))
                qT = qkT[64 * (h % 2) : 64 * (h % 2) + 64, (h // 2) * T : (h // 2 + 1) * T]
                kT = qkT[64 * (h % 2) : 64 * (h % 2) + 64, (2 + h // 2) * T : (3 + h // 2) * T]
                oT_ps = ps.tile([65, 512], F32, tag="ot", bufs=2)
                n_kj = (sc + 1) * 4
                npairs = n_kj // 2

                def pair_layout(p):
                    # [(bank_off, q_off, cols, kj), ...]; pack both k tiles
                    # into one PSUM bank when their columns fit (saves exp
                    # span and a bank)
                    kj0, kj1 = 2 * p, 2 * p + 1
                    qo0 = max(0, kj0 * 128 - sc * 512)
                    qo1 = max(0, kj1 * 128 - sc * 512)
                    c0, c1 = 512 - qo0, 512 - qo1
                    if c0 + c1 <= 512:
                        return [(0, qo0, c0, kj0), (c0, qo1, c1, kj1)], c0 + c1
                    return [(0, qo0, c0, kj0), (512, qo1, c1, kj1)], 512 + c1

                def emit_s_pair(p):
                    layout, span = pair_layout(p)
                    one_bank = layout[1][0] < 512
                    st = ps.tile([128, 1024], F32, tag="smm", bufs=2)
                    pt = stage.tile([128, 1024], F16, tag="pt", bufs=4)
                    # per-PSUM-bank balanced start/stop: in the one_bank pack
                    # both k-tiles share a group (start zeroes the whole zero
                    # region, so the second k-tile's disjoint columns
                    # accumulate onto zeros); otherwise one group per bank
                    for idx, (boff, q_off, cols, kj) in enumerate(layout):
                        diag = kj >= sc * 4
                        first = idx == 0 or not one_bank
                        last_in_group = (not one_bank) or idx == 1
                        nc.tensor.matmul(
                            st[:, boff : boff + cols],
                            kT[:, kj * 128 : (kj + 1) * 128],
                            qT[:, sc * 512 + q_off : (sc + 1) * 512],
                            start=first,
                            stop=(not diag) and last_in_group,
                        )
                        if diag:
                            nc.tensor.matmul(
                                st[:, boff : boff + 128], mstat[:], mmov[:],
                                start=False, stop=last_in_group,
                            )
                    # one exp covering both halves (cols between valid ranges
                    # hold stale PSUM; the pt garbage there is never read)
                    nc.scalar.activation(
                        pt[:, :span], st[:, :span],
                        mybir.ActivationFunctionType.Exp,
                        scale=EXP_SCALE,
                    )
                    return pt, layout

                def emit_pv_pair(p, pt, layout):
                    for boff, q_off, cols, kj in layout:
                        vv = v_aug[:, (kj * HPC + h) * 65 : (kj * HPC + h + 1) * 65]
                        nc.tensor.matmul(
                            oT_ps[:, q_off:512],
                            vv,
                            pt[:, boff : boff + cols],
                            start=(kj == 0),
                            stop=(kj == n_kj - 1),
                        )

                # software pipeline: emit S(p+1) before PV(p) so PE always has
                # matmul work queued while exp(p) completes on ACT; fillers
                # (independent work units, one LIST per exp-wait slot) absorb
                # ACT-bound gaps; leftovers flush before the final PV so they
                # still precede the normalize chain
                fillers = [list(f) for f in (fillers or [])]
                pend = emit_s_pair(0)
                for p in range(1, npairs):
                    nxt = emit_s_pair(p)
                    if fillers:
                        for f in fillers.pop(0):
                            f()
                    emit_pv_pair(p - 1, *pend)
                    pend = nxt
                for fl in fillers:
                    for f in fl:
                        f()
                emit_pv_pair(npairs - 1, *pend)
                # normalize chain for this head (DVE + Pool), frees oT_ps;
                # rowsum scaled by 1/OS so oT carries a xOS pre-scale
                rs = stage.tile([1, 512], F32, tag="rs", bufs=4)
                recip = stage.tile([1, 512], F32, tag="recip", bufs=4)
                bc = stage.tile([64, 512], F32, tag="bc", bufs=4)
                if sc == 2 and h == 1:
                    # last head: pipeline the rowsum->recip->broadcast chain
                    # in column halves across ACT/DVE/Pool so the first fin
                    # closes unblock ~0.5us earlier
                    for hf in range(2):
                        hs = slice(hf * 256, (hf + 1) * 256)
                        nc.scalar.activation(
                            rs[:, hs], oT_ps[64:65, hs],
                            mybir.ActivationFunctionType.Copy, scale=1.0 / OS,
                        )
                        nc.vector.reciprocal_approx_fast(recip[:, hs], rs[:, hs])
                        nc.gpsimd.partition_broadcast(bc[:, hs], recip[:, hs])
                else:
                    nc.vector.tensor_scalar_mul(rs[:], oT_ps[64:65, :], 1.0 / OS)
                    nc.vector.reciprocal_approx_fast(recip[:], rs[:])
                    nc.gpsimd.partition_broadcast(bc[:], recip[:])
                o16 = oT_sb[sc % 2][
                    64 * (h % 2) : 64 * (h % 2) + 64,
                    (h // 2) * 512 : (h // 2 + 1) * 512,
                ]
                if sc == 2 and h == 1:
                    # last head: 128-col pieces so each fin close piece j
                    # unblocks as soon as its oT columns land
                    for q4 in range(4):
                        qs = slice(q4 * 128, (q4 + 1) * 128)
                        nc.vector.tensor_mul(
                            o16[:, qs], oT_ps[0:64, qs], bc[:, qs]
                        )
                else:
                    nc.vector.tensor_mul(o16, oT_ps[0:64, :], bc[:])
                if make_fp8:
                    # both on Pool: keeps the DVE queue clear for the next
                    # head's rs/recip/mul chain (Pool has slack and its
                    # in-queue delay hides under the next head's PV)
                    pr = 64 * (h % 2)
                    hi = oT8_r[sc % 2][pr : pr + 64, h // 2, 0, :]
                    lo = oT8_r[sc % 2][pr : pr + 64, h // 2, 1, :]
                    nc.gpsimd.tensor_copy(hi, o16)
                    nc.gpsimd.tensor_sub(lo, o16, hi)

            def emit_proj_piece(sc, j, evac_on_act=None, dmaq=None):
                # fp8 path: project rows [sc*512 + j*128, +128).
                # DVE evac: stt folds descale + bias. ACT evac: bias enters
                # PSUM via a K=1 ones x (bias/descale) matmul, then a scaled
                # Copy (Pool cannot read PSUM).
                PHASE_MARKS.append((f"proj({sc},{j})", len(nc.inst_map)))
                r0 = sc * 512 + j * 128
                o8 = oT8_r[sc % 2]
                jr = slice(j * 128, (j + 1) * 128)
                on_act = bool(evac_on_act)
                pst = stage.tile([128, 1024], F16, tag="pst", bufs=4)
                for nch in range(2):
                    nr = slice(nch * 512, (nch + 1) * 512)
                    pp = ps.tile([128, 512], F32, tag="pmm", bufs=2)
                    nc.tensor.matmul(
                        pp[:],
                        o8[:, :, 0, jr],        # (pair0 hi, pair1 hi)
                        wpa8_r[:, :, 1, nr],    # (pair0 hi, pair1 hi)
                        start=True, stop=False, perf_mode=DR,
                    )
                    if on_act:
                        nc.tensor.matmul(
                            pp[:], ones_col[:],
                            bpr[:, C + nch * 512 : C + (nch + 1) * 512],
                            start=False, stop=False,
                        )
                    for p in range(2):
                        nc.tensor.matmul(
                            pp[:],
                            o8[:, p, :, jr],      # (hi, lo)
                            wpa8_r[:, p, :, nr],  # (lo, hi)
                            start=False, stop=(p == 1), perf_mode=DR,
                        )
                    if on_act:
                        nc.scalar.activation(
                            pst[:, nr], pp[:], mybir.ActivationFunctionType.Copy,
                            scale=PROJ8_DESCALE,
                        )
                    else:
                        nc.vector.scalar_tensor_tensor(
                            out=pst[:, nr], in0=pp[:], scalar=PROJ8_DESCALE,
                            in1=bias_bc[:, nr],
                            op0=mybir.AluOpType.mult, op1=mybir.AluOpType.add,
                        )
                # spread partial-write issues across SEQ queues: each DMA
                # issue occupies its queue ~650ns and the tail needs several
                # in flight at once
                dq = dmaq if dmaq is not None else [nc.sync, nc.scalar][j % 2]
                dq.dma_start(partial_d[r0 : r0 + 128, :], pst[:])

            def emit_rs(gi):
                r0, r1 = RS_GROUPS[gi]
                nc.gpsimd.collective_compute(
                    "ReduceScatter",
                    mybir.AluOpType.add,
                    replica_groups=[[0, 1, 2, 3], [4, 5, 6, 7]],
                    ins=[partial_d[r0:r1, :]],
                    outs=[rsout_d[gi][:]],
                )

            def emit_out_copy(gi):
                # deferred to the tail: an out-copy waiting on its collective
                # must not sit in an in-order DMA queue ahead of partial
                # writes that later collectives depend on
                r0, r1 = RS_GROUPS[gi]
                og = sum((b1 - b0) // 4 for (b0, b1) in RS_GROUPS[:gi])
                ln4 = (r1 - r0) // 4
                # huge scheduling-floor: keeps the tile scheduler from
                # hoisting these RS-gated copies ahead of the tail partial
                # writes in the same queue (ordering hint only, no HW wait)
                with tc.tile_wait_until(1.0):
                    nc.sync.dma_start(out_part[og : og + ln4, :], rsout_d[gi][:])

            # ---------------- schedule ----------------
            # PE warmup: the tensor engine clock ramps with sustained use and
            # the first ~9us are DMA-bound anyway, so spin dependency-free
            # dummy matmuls to hit full p-state before real work arrives.
            warm = ps.tile([128, 1024], F32, tag="smm", bufs=2)
            for i in range(120):
                nc.tensor.matmul(
                    warm[:, (i % 8) * 128 : (i % 8 + 1) * 128],
                    mstat[:], mmov[:],
                    start=True, stop=True, skip_group_check=True,
                )

            # Sub-chunk order: sc0, sc1, sc3, sc2. The LAST chunk's
            # projection gates the final ReduceScatter, so the smallest
            # remaining triangle (sc2) goes last; fin2 is the f16 tail path.
            # fp8 matmuls outrun the input DMAs at the start, so only the
            # QK(tch0) units precede att0 and everything else fills
            # attention's exp-wait slots in DMA-arrival order.
            emit_qk2(1, 3, 0)
            emit_qk2(0, 2, 0)

            # att sc0; V(0..3) land between S and PV; everything att1
            # needs fills the rest of att0's exp-wait slots
            att0_fill = [
                [[lambda: emit_v(0), lambda: emit_v(1)],
                 [lambda: emit_v(2), lambda: emit_v(3)]],
                [[lambda: emit_v(4)], [lambda: emit_v(5)]],
                [[lambda: emit_v(6)], [lambda: emit_qk(1, 1)]],
                [[lambda: emit_v(7)], [lambda: emit_qk(3, 1)]],
            ]
            for k, h in enumerate((2, 3, 0, 1)):
                emit_att_head(0, h, att0_fill[k])

            # att sc1; x-window-2/3 units + sc0 proj (oT8[0] is stable from
            # here on) fill the slots
            att1_fill = [
                [[lambda: emit_qk(0, 1)], [lambda: emit_v(8)],
                 [lambda: emit_v(9)]],
                [[lambda: emit_qk(2, 1)], [lambda: emit_v(10)],
                 [lambda: emit_v(11)]],
                [[lambda: emit_proj_piece(0, 0)], [lambda: emit_v(12)],
                 [lambda: emit_v(13)]],
                [[lambda: emit_proj_piece(0, 1)], [lambda: emit_v(14)],
                 [lambda: emit_v(15)]],
            ]
            for k, h in enumerate((2, 3, 0, 1)):
                emit_att_head(1, h, att1_fill[k])

            # fin1: att3's own Q/K units lead (covering the last sc1 head's
            # normalize chain), then the sc1 projection (it reads oT8[1],
            # which att3 overwrites -- the WAR dep keeps the reads safe)
            emit_qk(1, 3)
            emit_qk(3, 3)
            emit_qk(3, 2)
            emit_proj_piece(1, 0)
            emit_proj_piece(1, 1)
            emit_proj_piece(1, 2)
            emit_proj_piece(1, 3)

            # att sc3 (biggest triangle): remaining K/Q units for its own
            # later heads + sc0's remaining proj (oT8[0] stays untouched)
            att3_fill = [
                [[lambda: emit_qk(2, 2)], [lambda: emit_qk(2, 3)],
                 [lambda: emit_qk(0, 3)]],
                [[lambda: emit_qk(0, 2)], [lambda: emit_proj_piece(0, 2)]],
                [[lambda: emit_proj_piece(0, 3)]],
                [[lambda: emit_rs(0)]],
            ]
            for k, h in enumerate((2, 3, 0, 1)):
                emit_att_head(3, h, att3_fill[k])
            # fin3 lead-in: one ready QK unit covers the last sc3 head's
            # chain, then the first sc3 proj pieces
            emit_qk(1, 2)
            emit_proj_piece(3, 0)
            emit_proj_piece(3, 1)

            # att sc2 (last): rest of fin3 + its rs early; sc2 skips the fp8
            # split (f16 tail path).
            att2_fill = {
                2: [[lambda: emit_proj_piece(3, 2)]],
                3: [[lambda: emit_proj_piece(3, 3)]],
                0: [[lambda: emit_rs(1)]],
            }
            for k, h in enumerate((2, 3, 0, 1)):
                emit_att_head(2, h, att2_fill.get(h), make_fp8=False)

            # fin2: f16 tail projection for sc2 (no bias matmuls on the DVE
            # pieces; bias + 1/OS descale folded into the stt evacuation)
            oT_cur = oT_sb[0]
            tail_q = [nc.gpsimd, nc.scalar, nc.sync, nc.sync]
            tail_act = [True, True, False, False]

            def fin_open(j, kind):
                # kind: "smm" -> one [128,1024] tile split in halves;
                # "pmm" -> two [128,512] tiles (attention is done, both
                # pools are free -- this lets 3 opens stay in flight)
                PHASE_MARKS.append((f"fin_open{j}", len(nc.inst_map)))
                if kind == "smm":
                    t = ps.tile([128, 1024], F32, tag="smm", bufs=2)
                    aps = [t[:, 0:512], t[:, 512:1024]]
                else:
                    aps = []
                    for _n in range(2):
                        fpp = ps.tile(
                            [128, 512], F32, tag="pmm", bufs=2, name=f"fpp{_n}"
                        )
                        aps.append(fpp[:])
                for nch in range(2):
                    nc.tensor.matmul(
                        aps[nch],
                        oT_cur[:, 512 + j * 128 : 512 + (j + 1) * 128],
                        wpa16[:, C + nch * 512 : C + (nch + 1) * 512],
                        start=True, stop=False,
                    )
                    if tail_act[j]:
                        nc.tensor.matmul(
                            aps[nch],
                            ones_col[:],
                            bpr[:, 2 * C + nch * 512 : 2 * C + (nch + 1) * 512],
                            start=False, stop=False,
                        )
                return aps

            def fin_close(j, aps):
                PHASE_MARKS.append((f"fin_close{j}", len(nc.inst_map)))
                for nch in range(2):
                    nc.tensor.matmul(
                        aps[nch],
                        oT_cur[:, j * 128 : (j + 1) * 128],
                        wpa16[:, nch * 512 : (nch + 1) * 512],
                        start=False, stop=True,
                    )
                pst = stage.tile([128, 1024], F16, tag="pst", bufs=4)
                for nch in range(2):
                    nr = slice(nch * 512, (nch + 1) * 512)
                    if tail_act[j]:
                        nc.scalar.activation(
                            pst[:, nr], aps[nch],
                            mybir.ActivationFunctionType.Copy,
                            scale=PROJ16_DESCALE,
                        )
                    else:
                        nc.vector.scalar_tensor_tensor(
                            out=pst[:, nr], in0=aps[nch], scalar=PROJ16_DESCALE,
                            in1=bias_bc[:, nr],
                            op0=mybir.AluOpType.mult, op1=mybir.AluOpType.add,
                        )
                r0 = 2 * 512 + j * 128
                tail_q[j].dma_start(partial_d[r0 : r0 + 128, :], pst[:])

            if DEBUG:
                nc.sync.dma_start(dbg["oT1"][:], oT_sb[1][:])
                nc.sync.dma_start(dbg["oT81"][:], oT8_sb[1][:])
                nc.sync.dma_start(dbg["qkT"][:], qkT[:])
            pps = [fin_open(0, "smm"), fin_open(1, "smm"), fin_open(2, "pmm")]
            fin_close(0, pps[0])
            pp3 = fin_open(3, "smm")
            fin_close(1, pps[1])
            fin_close(2, pps[2])
            fin_close(3, pp3)
            emit_rs(2)
            for gi in range(len(RS_GROUPS)):
                emit_out_copy(gi)
            if DEBUG:
                nc.sync.dma_start(dbg["partial"][:], partial_d[:])

    nc.finalize()
    return nc


_NC = None


def _get_nc():
    global _NC
    if _NC is None:
        _NC = _build()
    return _NC


def _perm_qkv(w):
    # (..., h*192 + t*64 + c) -> (..., t*256 + h*64 + c)
    s = w.shape[:-1]
    return np.ascontiguousarray(
        w.reshape(*s, HPC, 3, HD).swapaxes(-3, -2).reshape(*s, CG)
    )


def _hilo(a):
    hi = a.astype(E4)
    lo = (a - hi.astype(np.float32)).astype(E4)
    return hi, lo


def _make_in_maps(x, Wqkv, bqkv, Wproj, bproj):
    x = np.asarray(x, dtype=np.float32)
    Wqkv = np.asarray(Wqkv, dtype=np.float32)
    bqkv = np.asarray(bqkv, dtype=np.float32)
    Wproj = np.asarray(Wproj, dtype=np.float32)
    bproj = np.asarray(bproj, dtype=np.float32)

    in_maps = []
    for c in range(N_CORES):
        b, g = divmod(c, 4)
        # x8: [p, w, kc, e(hi,lo), t]
        xT = x[b].T * XS  # (C, T)
        xa = xT.reshape(KC, 128, 4, 512).transpose(1, 2, 0, 3)  # [p, w, kc, t]
        xh, xl = _hilo(xa)
        x8 = np.stack([xh, xl], axis=3).reshape(128, -1)
        # w8: [p, kc, e(lo,hi), m]
        wp_ = _perm_qkv(Wqkv[:, g * CG : (g + 1) * CG]) * WS
        wa = wp_.reshape(KC, 128, CG).transpose(1, 0, 2)  # [p, kc, m]
        wh, wl = _hilo(wa)
        w8 = np.stack([wl, wh], axis=2).reshape(128, -1)
        bq = _perm_qkv(bqkv[g * CG : (g + 1) * CG])
        qkb = np.ascontiguousarray(bq[:512].reshape(4, 128).T).astype(np.float32)
        vb = bq[512:768].reshape(1, 256).astype(np.float16)
        # wpa: [p, pair, c]
        wpa = np.zeros((128, 2, C), np.float32)
        for pair in range(2):
            wpa[:, pair] = Wproj[
                g * 256 + pair * 128 : g * 256 + (pair + 1) * 128, :
            ]
        ph, pl = _hilo(wpa * WS)
        wpa8 = np.stack([pl, ph], axis=2).reshape(128, -1)
        b0 = bproj if g == 0 else np.zeros(C, np.float32)
        bpr = np.concatenate([b0, b0 * OS * WS, b0 * OS]).reshape(1, 3 * C)
        in_maps.append(
            {
                "x8": x8,
                "w8": w8,
                "qkb": qkb,
                "vb": vb,
                "wpa8": wpa8,
                "wpa16": wpa.reshape(128, -1).astype(np.float16),
                "bpr": bpr.astype(np.float16),
            }
        )
    return in_maps


def _run(in_maps, trace=False):
    nc = _get_nc()
    return run_bass_kernel_spmd(nc, in_maps, list(range(N_CORES)), trace=trace)


def kernel(x, Wqkv, bqkv, Wproj, bproj):
    in_maps = _make_in_maps(x, Wqkv, bqkv, Wproj, bproj)
    res = _run(in_maps)
    out = np.empty((B, T, C), np.float32)
    for c in range(N_CORES):
        b, g = divmod(c, 4)
        op = res.results[c]["out_part"].astype(np.float32)
        og = 0
        for r0, r1 in RS_GROUPS:
            ln4 = (r1 - r0) // 4
            out[b, r0 + g * ln4 : r0 + (g + 1) * ln4, :] = op[og : og + ln4]
            og += ln4
    return out


# revision 78
# speedup vs baseline: 1.0058x; 1.0038x over previous
"""Multi-head causal self-attention (B=2, T=2048, C=1024, H=16) on 8 trn2 cores.

Sharding: data-parallel over batch (2) x tensor-parallel over heads (4 groups
of 4 heads). Core c handles batch b=c//4, head group g=c%4.

Key structure (per core):
  - QKV and output projections run in fp8e4m3 DoubleRow with hi+lo error
    compensation: every operand X is host- (or device-) split into
    X_hi = fp8(X), X_lo = fp8(X - X_hi); each 256-channel contraction uses
    3 DoubleRow matmuls (hi*hi packing two k-tiles, plus one cross-term
    matmul per k-tile computing hi*lo + lo*hi in its two slots) instead of
    2 f16 matmuls -> 4x per-row speed at 3/2 the instruction count = 2.67x,
    with quantization error compensated to ~1e-3. Operands are pre-scaled
    (x*8, W*64, oT*16) to clear the e4m3 subnormal floor; descale happens
    in the PSUM evacuation op.
  - Attention stays f16 in S^T orientation (k on partitions, q free): the
    exp on ACT (~58us) is the attention-phase floor, so cheaper PE matmuls
    there would buy nothing. Causal mask via a PE add-matmul (-200
    strict-upper-tri stationary x identity moving).
  - Rowsums from a ones-column appended to V; recip on DVE (scaled 1/16 so
    oT carries a x16 pre-scale for its fp8 split); partition_broadcast on
    Pool.
  - All projection bias adds are folded into the PSUM evacuation
    (scalar_tensor_tensor with descale scalar + partition-broadcast bias)
    on DVE/Pool -- no bias matmuls on PE.
  - Sub-chunk processing order is sc0, sc1, sc3, sc2: the final
    ReduceScatter is gated by the last chunk's projection, so the smallest
    remaining causal triangle (sc2) goes last; its projection stays f16
    (reads oT directly, skipping the fp8 hi/lo split latency) to keep the
    tail chain short. PSUM is split into 2x[128,1024] S-tile buffers and
    2x[128,512] buffers for QKV/proj units so fillers never contend with
    the attention pipeline; PE warms its p-state on dummy matmuls during
    the DMA-bound preamble.
"""

import os

import numpy as np
import ml_dtypes

import concourse.bacc as bacc
import concourse.mybir as mybir
import concourse.tile as tile
from concourse.bass_utils import run_bass_kernel_spmd

DEBUG = bool(int(os.environ.get("KERNEL_DEBUG", "0")))
PHASE_MARKS = []  # (label, first_instruction_index) for trace attribution

F32 = mybir.dt.float32
F16 = mybir.dt.float16
F8 = mybir.dt.float8e4
E4 = ml_dtypes.float8_e4m3
DR = mybir.MatmulPerfMode.DoubleRow

B, T, C, H = 2, 2048, 1024, 16
HPC = 4                 # heads per core
HD = 64                 # head dim
CG = HPC * 3 * HD       # 768 qkv cols per core
KC = 8                  # contraction chunks (128 channels each)
TT = T // 128           # 16 k tiles
NSC = T // 512          # 4 q sub-chunks
N_CORES = 8
EXP_SCALE = 0.125

XS = 8.0                # host pre-scale on x
WS = 64.0               # host pre-scale on Wqkv / Wproj
OS = 16.0               # device pre-scale on attention output (via recip)
QKV_DESCALE = 1.0 / (XS * WS)
PROJ8_DESCALE = 1.0 / (OS * WS)
PROJ16_DESCALE = 1.0 / OS

# reduce-scatter groups as (row_start, row_end) in COMPLETION order
# (sc0+sc1, then sc3, then sc2 last); each core keeps len/4 rows
RS_GROUPS = [(0, 1024), (1536, 2048), (1024, 1536)]


def _build():
    nc = bacc.Bacc(None, target_bir_lowering=False)

    # x8: [p, w(4), kc(8), e(hi,lo), t(512)] fp8
    x8_in = nc.dram_tensor("x8", [128, 4 * KC * 2 * 512], F8, kind="ExternalInput")
    # w8: [p, kc(8), e(lo,hi), m(768)] fp8
    w8_in = nc.dram_tensor("w8", [128, KC * 2 * CG], F8, kind="ExternalInput")
    qkb_in = nc.dram_tensor("qkb", [128, 4], F32, kind="ExternalInput")
    vb_in = nc.dram_tensor("vb", [1, 256], F16, kind="ExternalInput")
    # wpa8: [p, pair(2), e(lo,hi), c(1024)] fp8 (x64 scaled Wproj rows)
    wpa8_in = nc.dram_tensor("wpa8", [128, 2 * 2 * C], F8, kind="ExternalInput")
    # wpa16: [p, pair(2)*c] f16 unscaled Wproj rows (tail path)
    wpa16_in = nc.dram_tensor("wpa16", [128, 2 * C], F16, kind="ExternalInput")
    # bpr blocks along free dim: [bproj | bproj*OS*WS | bproj*OS]
    # (blocks 1,2 are added in PSUM via a K=1 matmul before a scaled ACT evac)
    bpr_in = nc.dram_tensor("bpr", [1, 3 * C], F16, kind="ExternalInput")
    out_part = nc.dram_tensor("out_part", [T // 4, C], F16, kind="ExternalOutput")

    partial_d = nc.dram_tensor("partial_d", [T, C], F16)
    dbg = {}
    if DEBUG:
        dbg["oT1"] = nc.dram_tensor("dbg_oT1", [128, 2 * 512], F16, kind="ExternalOutput")
        dbg["oT81"] = nc.dram_tensor("dbg_oT81", [128, 4 * 512], F8, kind="ExternalOutput")
        dbg["partial"] = nc.dram_tensor("dbg_partial", [T, C], F16, kind="ExternalOutput")
        dbg["qkT"] = nc.dram_tensor("dbg_qkT", [128, 4 * T], F16, kind="ExternalOutput")
    rsout_d = [
        nc.dram_tensor(f"rsout_d{i}", [(r1 - r0) // 4, C], F16)
        for i, (r0, r1) in enumerate(RS_GROUPS)
    ]

    with tile.TileContext(nc) as tc:
        with (
            tc.tile_pool(name="cpool", bufs=1) as cpool,
            tc.tile_pool(name="main", bufs=1) as main,
            tc.tile_pool(name="stage", bufs=1) as stage,
            tc.tile_pool(name="ps", bufs=1, space="PSUM") as ps,
        ):
            # ---------------- constants ----------------
            vb_bc = cpool.tile([128, 256], F16)
            bias_bc = cpool.tile([128, C], F16)
            ones_col = cpool.tile([1, 128], F16)
            nc.vector.memset(ones_col[:], 1.0)
            # mask stationary: mstat[f, p] = -200 where p > f else 0
            mstat = cpool.tile([128, 128], F16)
            nc.gpsimd.memset(mstat[:], -200.0)
            nc.gpsimd.affine_select(
                out=mstat[:], in_=mstat[:],
                compare_op=mybir.AluOpType.is_ge, fill=0.0,
                base=-1, pattern=[[1, 128]], channel_multiplier=-1,
            )
            # mask moving: identity
            mmov = cpool.tile([128, 128], F16)
            nc.gpsimd.memset(mmov[:], 0.0)
            nc.gpsimd.affine_select(
                out=mmov[:], in_=mmov[:],
                compare_op=mybir.AluOpType.not_equal, fill=1.0,
                base=0, pattern=[[-1, 128]], channel_multiplier=1,
            )

            # ---------------- persistent tensors ----------------
            x8 = main.tile([128, 4 * KC * 2 * 512], F8)
            w8 = main.tile([128, KC * 2 * CG], F8)
            qkb = main.tile([128, 4], F32)
            vb = main.tile([1, 256], F16)
            wpa8 = main.tile([128, 2 * 2 * C], F8)
            wpa16 = main.tile([128, 2 * C], F16)
            bpr = main.tile([1, 3 * C], F16)
            qkT = main.tile([128, 4 * T], F16)             # [Q01;Q23;K01;K23] x T
            v_aug = main.tile([128, TT * HPC * 65], F16)   # per (tt,h): 64 V + ones col
            # f16 attention outs, [pair][512] col layout (x16 pre-scaled);
            # head h lives at partitions 64*(h%2).., column block (h//2)*512
            oT_sb = [
                main.tile([128, 2 * 512], F16, name=f"oT_sb{i}") for i in range(2)
            ]
            # fp8 hi/lo split of oT for the fp8 projection path:
            # [p, pair(2), e(hi,lo), 512]
            oT8_sb = [
                main.tile([128, 2 * 2 * 512], F8, name=f"oT8_sb{i}") for i in range(2)
            ]

            nc.vector.memset(v_aug[:], 1.0)  # ones columns give softmax rowsums

            # ---------------- views ----------------
            w8_r = w8[:].rearrange("p (kc e m) -> p kc e m", kc=KC, e=2)
            w8_in_r = w8_in[:].rearrange("p (kc e m) -> p kc e m", kc=KC, e=2)
            wpa8_r = wpa8[:].rearrange("p (pr e c) -> p pr e c", pr=2, e=2)
            oT8_r = [
                t[:].rearrange("p (pr e c) -> p pr e c", pr=2, e=2) for t in oT8_sb
            ]

            def x8_w(w):
                # [128, kc, e, 512] view of window w
                return x8[:, w * 8192 : (w + 1) * 8192].rearrange(
                    "p (kc e t) -> p kc e t", kc=KC, e=2
                )

            # ---------------- input DMAs ----------------
            # DMA transfers serialize on the DMA-engine pool, so order them
            # by what gates compute: x window 0 + QK weights interleaved per
            # kc pair (attention sc0's S matmuls are the longest dependency
            # chain), V weights next (PV trails S by ~1.5us), then the later
            # x windows.
            for kp in range(4):
                nc.sync.dma_start(
                    x8[:, kp * 2048 : (kp + 1) * 2048],
                    x8_in[:, kp * 2048 : (kp + 1) * 2048],
                )
                nc.sync.dma_start(
                    w8_r[:, 2 * kp : 2 * kp + 2, :, 0:512],
                    w8_in_r[:, 2 * kp : 2 * kp + 2, :, 0:512],
                )
            nc.sync.dma_start(vb[:], vb_in[:])
            nc.sync.dma_start(qkb[:], qkb_in[:])
            nc.gpsimd.partition_broadcast(vb_bc[:], vb[:])
            nc.sync.dma_start(w8_r[:, 0:4, :, 512:768], w8_in_r[:, 0:4, :, 512:768])
            nc.sync.dma_start(w8_r[:, 4:8, :, 512:768], w8_in_r[:, 4:8, :, 512:768])
            nc.sync.dma_start(x8[:, 8192:16384], x8_in[:, 8192:16384])      # w1
            nc.scalar.dma_start(wpa8[:], wpa8_in[:])
            nc.scalar.dma_start(bpr[:], bpr_in[:])
            nc.gpsimd.partition_broadcast(bias_bc[:], bpr[:, 0:C])
            nc.sync.dma_start(x8[:, 16384:24576], x8_in[:, 16384:24576])    # w2
            nc.sync.dma_start(x8[:, 24576:32768], x8_in[:, 24576:32768])    # w3
            nc.scalar.dma_start(wpa16[:], wpa16_in[:])

            # ---------------- emit helpers ----------------
            def mm_pairs(pp_slice, stat_of, mov_of):
                """Emit the compensated fp8 matmul group: for each kc pair,
                hi*hi (2 slots = the 2 k-tiles), then a cross matmul per
                k-tile (slots = hi*lo + lo*hi)."""
                n = KC // 2
                for p in range(n):
                    c0 = 2 * p
                    nc.tensor.matmul(
                        pp_slice, stat_of(c0, None), mov_of(c0, None),
                        start=(p == 0), stop=False, perf_mode=DR,
                    )
                    for c in (c0, c0 + 1):
                        nc.tensor.matmul(
                            pp_slice, stat_of(None, c), mov_of(None, c),
                            start=False, stop=(c == KC - 1), perf_mode=DR,
                        )

            def emit_v(tt):
                PHASE_MARKS.append((f"V{tt}", len(nc.inst_map)))
                w, tloc = divmod(tt, 4)
                xw = x8_w(w)
                tr = slice(tloc * 128, (tloc + 1) * 128)
                pp = ps.tile([128, 512], F32, tag="pmm", bufs=2)

                def stat(pair_c0, cross_c):
                    if pair_c0 is not None:  # hi*hi: slots (kc0,hi),(kc1,hi)
                        return xw[:, pair_c0 : pair_c0 + 2, 0, tr]
                    return xw[:, cross_c, :, tr]  # (hi,lo)

                def mov(pair_c0, cross_c):
                    if pair_c0 is not None:  # slots (kc0,hi),(kc1,hi)
                        return w8_r[:, pair_c0 : pair_c0 + 2, 1, 512:768]
                    return w8_r[:, cross_c, :, 512:768]  # (lo,hi)

                mm_pairs(pp[:, 0:256], stat, mov)
                vt = v_aug[:, tt * HPC * 65 : (tt + 1) * HPC * 65].rearrange(
                    "p (h c) -> p h c", c=65
                )[:, :, 0:64]
                nc.vector.scalar_tensor_tensor(
                    out=vt,
                    in0=pp[:, 0:256].rearrange("p (h c) -> p h c", c=64),
                    scalar=QKV_DESCALE,
                    in1=vb_bc[:].rearrange("p (h c) -> p h c", c=64),
                    op0=mybir.AluOpType.mult,
                    op1=mybir.AluOpType.add,
                )

            def emit_qk(i, tch):
                PHASE_MARKS.append((f"QK({i},{tch})", len(nc.inst_map)))
                xw = x8_w(tch)
                ir = slice(i * 128, (i + 1) * 128)
                pp0 = ps.tile([128, 512], F32, tag="pmm", bufs=2)
                pp = pp0[:]

                def stat(pair_c0, cross_c):
                    if pair_c0 is not None:
                        return w8_r[:, pair_c0 : pair_c0 + 2, 1, ir]
                    return w8_r[:, cross_c, :, ir]

                def mov(pair_c0, cross_c):
                    if pair_c0 is not None:
                        return xw[:, pair_c0 : pair_c0 + 2, 0, :]
                    return xw[:, cross_c, :, :]

                mm_pairs(pp, stat, mov)
                dst = qkT[:, i * T + tch * 512 : i * T + (tch + 1) * 512]
                # DVE, not ACT: evacuations on ACT would queue ahead of the
                # attention exps and stretch the S-tile free latency
                nc.vector.tensor_scalar(
                    out=dst, in0=pp, scalar1=QKV_DESCALE,
                    scalar2=qkb[:, i : i + 1],
                    op0=mybir.AluOpType.mult, op1=mybir.AluOpType.add,
                )

            def emit_qk2(ia, ib, tch):
                # two QK units interleaved by kc pair: during the DMA-bound
                # preamble neither unit serializes behind the other's stall,
                # so both evacuate as soon as the last kc pieces land
                PHASE_MARKS.append((f"QK2({ia},{ib},{tch})", len(nc.inst_map)))
                xw = x8_w(tch)
                pps = {}
                for i in (ia, ib):
                    pps[i] = ps.tile(
                        [128, 512], F32, tag="pmm", bufs=2, name=f"qk2_{i}"
                    )
                for p in range(KC // 2):
                    c0 = 2 * p
                    for i in (ia, ib):
                        ir = slice(i * 128, (i + 1) * 128)
                        nc.tensor.matmul(
                            pps[i][:], w8_r[:, c0 : c0 + 2, 1, ir],
                            xw[:, c0 : c0 + 2, 0, :],
                            start=(p == 0), stop=False, perf_mode=DR,
                        )
                        for c in (c0, c0 + 1):
                            nc.tensor.matmul(
                                pps[i][:], w8_r[:, c, :, ir], xw[:, c, :, :],
                                start=False, stop=(c == KC - 1), perf_mode=DR,
                            )
                for i in (ia, ib):
                    dst = qkT[:, i * T + tch * 512 : i * T + (tch + 1) * 512]
                    nc.vector.tensor_scalar(
                        out=dst, in0=pps[i][:], scalar1=QKV_DESCALE,
                        scalar2=qkb[:, i : i + 1],
                        op0=mybir.AluOpType.mult, op1=mybir.AluOpType.add,
                    )

            def emit_att_head(sc, h, fillers=None, make_fp8=True):
                PHASE_MARKS.append((f"att{sc}h{h}", len(nc.inst_map)))
                qT = qkT[64 * (h % 2) : 64 * (h % 2) + 64, (h // 2) * T : (h // 2 + 1) * T]
                kT = qkT[64 * (h % 2) : 64 * (h % 2) + 64, (2 + h // 2) * T : (3 + h // 2) * T]
                oT_ps = ps.tile([65, 512], F32, tag="ot", bufs=2)
                n_kj = (sc + 1) * 4
                npairs = n_kj // 2

                def pair_layout(p):
                    # [(bank_off, q_off, cols, kj), ...]; pack both k tiles
                    # into one PSUM bank when their columns fit (saves exp
                    # span and a bank)
                    kj0, kj1 = 2 * p, 2 * p + 1
                    qo0 = max(0, kj0 * 128 - sc * 512)
                    qo1 = max(0, kj1 * 128 - sc * 512)
                    c0, c1 = 512 - qo0, 512 - qo1
                    if c0 + c1 <= 512:
                        return [(0, qo0, c0, kj0), (c0, qo1, c1, kj1)], c0 + c1
                    return [(0, qo0, c0, kj0), (512, qo1, c1, kj1)], 512 + c1

                def emit_s_pair(p):
                    layout, span = pair_layout(p)
                    one_bank = layout[1][0] < 512
                    # heads with no fillers: first pair via two pmm
                    # half-tiles (idle there) so the head start skips the
                    # previous head's trailing-exp smm WAR
                    if p == 0 and npairs >= 5 and not fillers:
                        pt = stage.tile([128, 1024], F16, tag="pt", bufs=4)
                        for idx, (boff, q_off, cols, kj) in enumerate(layout):
                            s5 = ps.tile(
                                [128, 512], F32, tag="pmm", bufs=2,
                                name=f"s5_{idx}",
                            )
                            nc.tensor.matmul(
                                s5[:, 0:cols],
                                kT[:, kj * 128 : (kj + 1) * 128],
                                qT[:, sc * 512 + q_off : (sc + 1) * 512],
                                start=True, stop=True,
                            )
                            nc.scalar.activation(
                                pt[:, boff : boff + cols], s5[:, 0:cols],
                                mybir.ActivationFunctionType.Exp,
                                scale=EXP_SCALE,
                            )
                        return pt, layout
                    st = ps.tile([128, 1024], F32, tag="smm", bufs=2)
                    pt = stage.tile([128, 1024], F16, tag="pt", bufs=4)
                    # per-PSUM-bank balanced start/stop: in the one_bank pack
                    # both k-tiles share a group (start zeroes the whole zero
                    # region, so the second k-tile's disjoint columns
                    # accumulate onto zeros); otherwise one group per bank
                    for idx, (boff, q_off, cols, kj) in enumerate(layout):
                        diag = kj >= sc * 4
                        first = idx == 0 or not one_bank
                        last_in_group = (not one_bank) or idx == 1
                        nc.tensor.matmul(
                            st[:, boff : boff + cols],
                            kT[:, kj * 128 : (kj + 1) * 128],
                            qT[:, sc * 512 + q_off : (sc + 1) * 512],
                            start=first,
                            stop=(not diag) and last_in_group,
                        )
                        if diag:
                            nc.tensor.matmul(
                                st[:, boff : boff + 128], mstat[:], mmov[:],
                                start=False, stop=last_in_group,
                            )
                    # one exp covering both halves (cols between valid ranges
                    # hold stale PSUM; the pt garbage there is never read)
                    nc.scalar.activation(
                        pt[:, :span], st[:, :span],
                        mybir.ActivationFunctionType.Exp,
                        scale=EXP_SCALE,
                    )
                    return pt, layout

                def emit_pv_pair(p, pt, layout):
                    for boff, q_off, cols, kj in layout:
                        vv = v_aug[:, (kj * HPC + h) * 65 : (kj * HPC + h + 1) * 65]
                        nc.tensor.matmul(
                            oT_ps[:, q_off:512],
                            vv,
                            pt[:, boff : boff + cols],
                            start=(kj == 0),
                            stop=(kj == n_kj - 1),
                        )

                # software pipeline: emit S(p+1) before PV(p) so PE always has
                # matmul work queued while exp(p) completes on ACT; fillers
                # (independent work units, one LIST per exp-wait slot) absorb
                # ACT-bound gaps; leftovers flush before the final PV so they
                # still precede the normalize chain
                fillers = [list(f) for f in (fillers or [])]
                pend = emit_s_pair(0)
                for p in range(1, npairs):
                    nxt = emit_s_pair(p)
                    if fillers:
                        for f in fillers.pop(0):
                            f()
                    emit_pv_pair(p - 1, *pend)
                    pend = nxt
                for fl in fillers:
                    for f in fl:
                        f()
                emit_pv_pair(npairs - 1, *pend)
                # normalize chain for this head (DVE + Pool), frees oT_ps;
                # rowsum scaled by 1/OS so oT carries a xOS pre-scale
                rs = stage.tile([1, 512], F32, tag="rs", bufs=4)
                recip = stage.tile([1, 512], F32, tag="recip", bufs=4)
                bc = stage.tile([64, 512], F32, tag="bc", bufs=4)
                if sc == 2 and h == 1:
                    # last head: pipeline the rowsum->recip->broadcast chain
                    # in column halves across ACT/DVE/Pool so the first fin
                    # closes unblock ~0.5us earlier
                    for hf in range(2):
                        hs = slice(hf * 256, (hf + 1) * 256)
                        nc.scalar.activation(
                            rs[:, hs], oT_ps[64:65, hs],
                            mybir.ActivationFunctionType.Copy, scale=1.0 / OS,
                        )
                        nc.vector.reciprocal_approx_fast(recip[:, hs], rs[:, hs])
                        nc.gpsimd.partition_broadcast(bc[:, hs], recip[:, hs])
                else:
                    nc.vector.tensor_scalar_mul(rs[:], oT_ps[64:65, :], 1.0 / OS)
                    nc.vector.reciprocal_approx_fast(recip[:], rs[:])
                    nc.gpsimd.partition_broadcast(bc[:], recip[:])
                o16 = oT_sb[sc % 2][
                    64 * (h % 2) : 64 * (h % 2) + 64,
                    (h // 2) * 512 : (h // 2 + 1) * 512,
                ]
                if sc == 2 and h == 1:
                    # last head: 128-col pieces so each fin close piece j
                    # unblocks as soon as its oT columns land
                    for q4 in range(4):
                        qs = slice(q4 * 128, (q4 + 1) * 128)
                        nc.vector.tensor_mul(
                            o16[:, qs], oT_ps[0:64, qs], bc[:, qs]
                        )
                else:
                    nc.vector.tensor_mul(o16, oT_ps[0:64, :], bc[:])
                if make_fp8:
                    # both on Pool: keeps the DVE queue clear for the next
                    # head's rs/recip/mul chain (Pool has slack and its
                    # in-queue delay hides under the next head's PV)
                    pr = 64 * (h % 2)
                    hi = oT8_r[sc % 2][pr : pr + 64, h // 2, 0, :]
                    lo = oT8_r[sc % 2][pr : pr + 64, h // 2, 1, :]
                    nc.gpsimd.tensor_copy(hi, o16)
                    nc.gpsimd.tensor_sub(lo, o16, hi)

            def emit_proj_piece(sc, j, evac_on_act=None, dmaq=None):
                # fp8 path: project rows [sc*512 + j*128, +128).
                # DVE evac: stt folds descale + bias. ACT evac: bias enters
                # PSUM via a K=1 ones x (bias/descale) matmul, then a scaled
                # Copy (Pool cannot read PSUM).
                PHASE_MARKS.append((f"proj({sc},{j})", len(nc.inst_map)))
                r0 = sc * 512 + j * 128
                o8 = oT8_r[sc % 2]
                jr = slice(j * 128, (j + 1) * 128)
                on_act = bool(evac_on_act)
                pst = stage.tile([128, 1024], F16, tag="pst", bufs=4)
                for nch in range(2):
                    nr = slice(nch * 512, (nch + 1) * 512)
                    pp = ps.tile([128, 512], F32, tag="pmm", bufs=2)
                    nc.tensor.matmul(
                        pp[:],
                        o8[:, :, 0, jr],        # (pair0 hi, pair1 hi)
                        wpa8_r[:, :, 1, nr],    # (pair0 hi, pair1 hi)
                        start=True, stop=False, perf_mode=DR,
                    )
                    if on_act:
                        nc.tensor.matmul(
                            pp[:], ones_col[:],
                            bpr[:, C + nch * 512 : C + (nch + 1) * 512],
                            start=False, stop=False,
                        )
                    for p in range(2):
                        nc.tensor.matmul(
                            pp[:],
                            o8[:, p, :, jr],      # (hi, lo)
                            wpa8_r[:, p, :, nr],  # (lo, hi)
                            start=False, stop=(p == 1), perf_mode=DR,
                        )
                    if on_act:
                        nc.scalar.activation(
                            pst[:, nr], pp[:], mybir.ActivationFunctionType.Copy,
                            scale=PROJ8_DESCALE,
                        )
                    else:
                        nc.vector.scalar_tensor_tensor(
                            out=pst[:, nr], in0=pp[:], scalar=PROJ8_DESCALE,
                            in1=bias_bc[:, nr],
                            op0=mybir.AluOpType.mult, op1=mybir.AluOpType.add,
                        )
                # spread partial-write issues across SEQ queues: each DMA
                # issue occupies its queue ~650ns and the tail needs several
                # in flight at once
                dq = dmaq if dmaq is not None else [nc.sync, nc.scalar][j % 2]
                dq.dma_start(partial_d[r0 : r0 + 128, :], pst[:])

            def emit_rs(gi):
                r0, r1 = RS_GROUPS[gi]
                nc.gpsimd.collective_compute(
                    "ReduceScatter",
                    mybir.AluOpType.add,
                    replica_groups=[[0, 1, 2, 3], [4, 5, 6, 7]],
                    ins=[partial_d[r0:r1, :]],
                    outs=[rsout_d[gi][:]],
                )

            def emit_out_copy(gi):
                # deferred to the tail: an out-copy waiting on its collective
                # must not sit in an in-order DMA queue ahead of partial
                # writes that later collectives depend on
                r0, r1 = RS_GROUPS[gi]
                og = sum((b1 - b0) // 4 for (b0, b1) in RS_GROUPS[:gi])
                ln4 = (r1 - r0) // 4
                # huge scheduling-floor: keeps the tile scheduler from
                # hoisting these RS-gated copies ahead of the tail partial
                # writes in the same queue (ordering hint only, no HW wait)
                with tc.tile_wait_until(1.0):
                    nc.sync.dma_start(out_part[og : og + ln4, :], rsout_d[gi][:])

            # ---------------- schedule ----------------
            # PE warmup: the tensor engine clock ramps with sustained use and
            # the first ~9us are DMA-bound anyway, so spin dependency-free
            # dummy matmuls to hit full p-state before real work arrives.
            warm = ps.tile([128, 1024], F32, tag="smm", bufs=2)
            for i in range(120):
                nc.tensor.matmul(
                    warm[:, (i % 8) * 128 : (i % 8 + 1) * 128],
                    mstat[:], mmov[:],
                    start=True, stop=True, skip_group_check=True,
                )

            # Sub-chunk order: sc0, sc1, sc3, sc2. The LAST chunk's
            # projection gates the final ReduceScatter, so the smallest
            # remaining triangle (sc2) goes last; fin2 is the f16 tail path.
            # fp8 matmuls outrun the input DMAs at the start, so only the
            # QK(tch0) units precede att0 and everything else fills
            # attention's exp-wait slots in DMA-arrival order.
            emit_qk2(1, 3, 0)
            emit_qk2(0, 2, 0)

            # att sc0; V(0..3) land between S and PV; everything att1
            # needs fills the rest of att0's exp-wait slots
            att0_fill = [
                [[lambda: emit_v(0), lambda: emit_v(1)],
                 [lambda: emit_v(2), lambda: emit_v(3)]],
                [[lambda: emit_v(4)], [lambda: emit_v(5)]],
                [[lambda: emit_v(6)], [lambda: emit_qk(1, 1)]],
                [[lambda: emit_v(7)], [lambda: emit_qk(3, 1)]],
            ]
            for k, h in enumerate((2, 3, 0, 1)):
                emit_att_head(0, h, att0_fill[k])

            # att sc1; x-window-2/3 units + sc0 proj (oT8[0] is stable from
            # here on) fill the slots
            att1_fill = [
                [[lambda: emit_qk(0, 1)], [lambda: emit_v(8)],
                 [lambda: emit_v(9)]],
                [[lambda: emit_qk(2, 1)], [lambda: emit_v(10)],
                 [lambda: emit_v(11)]],
                [[lambda: emit_proj_piece(0, 0)], [lambda: emit_v(12)],
                 [lambda: emit_v(13)]],
                [[lambda: emit_proj_piece(0, 1)], [lambda: emit_v(14)],
                 [lambda: emit_v(15)]],
            ]
            for k, h in enumerate((2, 3, 0, 1)):
                emit_att_head(1, h, att1_fill[k])

            # fin1: att3's own Q/K units lead (covering the last sc1 head's
            # normalize chain), then the sc1 projection (it reads oT8[1],
            # which att3 overwrites -- the WAR dep keeps the reads safe)
            emit_qk(1, 3)
            emit_qk(3, 3)
            emit_qk(3, 2)
            emit_proj_piece(1, 0)
            emit_proj_piece(1, 1)
            emit_proj_piece(1, 2)
            emit_proj_piece(1, 3)

            # att sc3 (biggest triangle): remaining K/Q units for its own
            # later heads + sc0's remaining proj (oT8[0] stays untouched)
            att3_fill = [
                [[lambda: emit_qk(2, 2)], [lambda: emit_qk(2, 3)],
                 [lambda: emit_qk(0, 3)]],
                [[lambda: emit_qk(0, 2)], [lambda: emit_proj_piece(0, 2)]],
                [[lambda: emit_proj_piece(0, 3)]],
                [[lambda: emit_rs(0)]],
            ]
            for k, h in enumerate((2, 3, 0, 1)):
                emit_att_head(3, h, att3_fill[k])
            # fin3 lead-in: one ready QK unit covers the last sc3 head's
            # chain, then the first sc3 proj pieces
            emit_qk(1, 2)
            emit_proj_piece(3, 0)
            emit_proj_piece(3, 1)

            # att sc2 (last): rest of fin3 + its rs early; sc2 skips the fp8
            # split (f16 tail path).
            att2_fill = {
                2: [[lambda: emit_proj_piece(3, 2)]],
                3: [[lambda: emit_proj_piece(3, 3)]],
                0: [[lambda: emit_rs(1)]],
            }
            for k, h in enumerate((2, 3, 0, 1)):
                emit_att_head(2, h, att2_fill.get(h), make_fp8=False)

            # fin2: f16 tail projection for sc2 (no bias matmuls on the DVE
            # pieces; bias + 1/OS descale folded into the stt evacuation)
            oT_cur = oT_sb[0]
            tail_q = [nc.gpsimd, nc.scalar, nc.sync, nc.sync]
            tail_act = [True, True, False, False]

            def fin_open(j, kind):
                # kind: "smm" -> one [128,1024] tile split in halves;
                # "pmm" -> two [128,512] tiles (attention is done, both
                # pools are free -- this lets 3 opens stay in flight)
                PHASE_MARKS.append((f"fin_open{j}", len(nc.inst_map)))
                if kind == "smm":
                    t = ps.tile([128, 1024], F32, tag="smm", bufs=2)
                    aps = [t[:, 0:512], t[:, 512:1024]]
                else:
                    aps = []
                    for _n in range(2):
                        fpp = ps.tile(
                            [128, 512], F32, tag="pmm", bufs=2, name=f"fpp{_n}"
                        )
                        aps.append(fpp[:])
                for nch in range(2):
                    nc.tensor.matmul(
                        aps[nch],
                        oT_cur[:, 512 + j * 128 : 512 + (j + 1) * 128],
                        wpa16[:, C + nch * 512 : C + (nch + 1) * 512],
                        start=True, stop=False,
                    )
                    if tail_act[j]:
                        nc.tensor.matmul(
                            aps[nch],
                            ones_col[:],
                            bpr[:, 2 * C + nch * 512 : 2 * C + (nch + 1) * 512],
                            start=False, stop=False,
                        )
                return aps

            def fin_close(j, aps):
                PHASE_MARKS.append((f"fin_close{j}", len(nc.inst_map)))
                for nch in range(2):
                    nc.tensor.matmul(
                        aps[nch],
                        oT_cur[:, j * 128 : (j + 1) * 128],
                        wpa16[:, nch * 512 : (nch + 1) * 512],
                        start=False, stop=True,
                    )
                pst = stage.tile([128, 1024], F16, tag="pst", bufs=4)
                for nch in range(2):
                    nr = slice(nch * 512, (nch + 1) * 512)
                    if tail_act[j]:
                        nc.scalar.activation(
                            pst[:, nr], aps[nch],
                            mybir.ActivationFunctionType.Copy,
                            scale=PROJ16_DESCALE,
                        )
                    else:
                        nc.vector.scalar_tensor_tensor(
                            out=pst[:, nr], in0=aps[nch], scalar=PROJ16_DESCALE,
                            in1=bias_bc[:, nr],
                            op0=mybir.AluOpType.mult, op1=mybir.AluOpType.add,
                        )
                r0 = 2 * 512 + j * 128
                tail_q[j].dma_start(partial_d[r0 : r0 + 128, :], pst[:])

            if DEBUG:
                nc.sync.dma_start(dbg["oT1"][:], oT_sb[1][:])
                nc.sync.dma_start(dbg["oT81"][:], oT8_sb[1][:])
                nc.sync.dma_start(dbg["qkT"][:], qkT[:])
            pps = [fin_open(0, "smm"), fin_open(1, "smm"), fin_open(2, "pmm")]
            fin_close(0, pps[0])
            pp3 = fin_open(3, "smm")
            fin_close(1, pps[1])
            fin_close(2, pps[2])
            fin_close(3, pp3)
            emit_rs(2)
            for gi in range(len(RS_GROUPS)):
                emit_out_copy(gi)
            if DEBUG:
                nc.sync.dma_start(dbg["partial"][:], partial_d[:])

    nc.finalize()
    return nc


_NC = None


def _get_nc():
    global _NC
    if _NC is None:
        _NC = _build()
    return _NC


def _perm_qkv(w):
    # (..., h*192 + t*64 + c) -> (..., t*256 + h*64 + c)
    s = w.shape[:-1]
    return np.ascontiguousarray(
        w.reshape(*s, HPC, 3, HD).swapaxes(-3, -2).reshape(*s, CG)
    )


def _hilo(a):
    hi = a.astype(E4)
    lo = (a - hi.astype(np.float32)).astype(E4)
    return hi, lo


def _make_in_maps(x, Wqkv, bqkv, Wproj, bproj):
    x = np.asarray(x, dtype=np.float32)
    Wqkv = np.asarray(Wqkv, dtype=np.float32)
    bqkv = np.asarray(bqkv, dtype=np.float32)
    Wproj = np.asarray(Wproj, dtype=np.float32)
    bproj = np.asarray(bproj, dtype=np.float32)

    in_maps = []
    for c in range(N_CORES):
        b, g = divmod(c, 4)
        # x8: [p, w, kc, e(hi,lo), t]
        xT = x[b].T * XS  # (C, T)
        xa = xT.reshape(KC, 128, 4, 512).transpose(1, 2, 0, 3)  # [p, w, kc, t]
        xh, xl = _hilo(xa)
        x8 = np.stack([xh, xl], axis=3).reshape(128, -1)
        # w8: [p, kc, e(lo,hi), m]
        wp_ = _perm_qkv(Wqkv[:, g * CG : (g + 1) * CG]) * WS
        wa = wp_.reshape(KC, 128, CG).transpose(1, 0, 2)  # [p, kc, m]
        wh, wl = _hilo(wa)
        w8 = np.stack([wl, wh], axis=2).reshape(128, -1)
        bq = _perm_qkv(bqkv[g * CG : (g + 1) * CG])
        qkb = np.ascontiguousarray(bq[:512].reshape(4, 128).T).astype(np.float32)
        vb = bq[512:768].reshape(1, 256).astype(np.float16)
        # wpa: [p, pair, c]
        wpa = np.zeros((128, 2, C), np.float32)
        for pair in range(2):
            wpa[:, pair] = Wproj[
                g * 256 + pair * 128 : g * 256 + (pair + 1) * 128, :
            ]
        ph, pl = _hilo(wpa * WS)
        wpa8 = np.stack([pl, ph], axis=2).reshape(128, -1)
        b0 = bproj if g == 0 else np.zeros(C, np.float32)
        bpr = np.concatenate([b0, b0 * OS * WS, b0 * OS]).reshape(1, 3 * C)
        in_maps.append(
            {
                "x8": x8,
                "w8": w8,
                "qkb": qkb,
                "vb": vb,
                "wpa8": wpa8,
                "wpa16": wpa.reshape(128, -1).astype(np.float16),
                "bpr": bpr.astype(np.float16),
            }
        )
    return in_maps


def _run(in_maps, trace=False):
    nc = _get_nc()
    return run_bass_kernel_spmd(nc, in_maps, list(range(N_CORES)), trace=trace)


def kernel(x, Wqkv, bqkv, Wproj, bproj):
    in_maps = _make_in_maps(x, Wqkv, bqkv, Wproj, bproj)
    res = _run(in_maps)
    out = np.empty((B, T, C), np.float32)
    for c in range(N_CORES):
        b, g = divmod(c, 4)
        op = res.results[c]["out_part"].astype(np.float32)
        og = 0
        for r0, r1 in RS_GROUPS:
            ln4 = (r1 - r0) // 4
            out[b, r0 + g * ln4 : r0 + (g + 1) * ln4, :] = op[og : og + ln4]
            og += ln4
    return out


# revision 84
# speedup vs baseline: 1.0146x; 1.0088x over previous
"""Multi-head causal self-attention (B=2, T=2048, C=1024, H=16) on 8 trn2 cores.

Sharding: data-parallel over batch (2) x tensor-parallel over heads (4 groups
of 4 heads). Core c handles batch b=c//4, head group g=c%4.

Key structure (per core):
  - QKV and output projections run in fp8e4m3 DoubleRow with hi+lo error
    compensation: every operand X is host- (or device-) split into
    X_hi = fp8(X), X_lo = fp8(X - X_hi); each 256-channel contraction uses
    3 DoubleRow matmuls (hi*hi packing two k-tiles, plus one cross-term
    matmul per k-tile computing hi*lo + lo*hi in its two slots) instead of
    2 f16 matmuls -> 4x per-row speed at 3/2 the instruction count = 2.67x,
    with quantization error compensated to ~1e-3. Operands are pre-scaled
    (x*8, W*64, oT*16) to clear the e4m3 subnormal floor; descale happens
    in the PSUM evacuation op.
  - Attention stays f16 in S^T orientation (k on partitions, q free): the
    exp on ACT (~58us) is the attention-phase floor, so cheaper PE matmuls
    there would buy nothing. Causal mask via a PE add-matmul (-200
    strict-upper-tri stationary x identity moving).
  - Rowsums from a ones-column appended to V; recip on DVE (scaled 1/16 so
    oT carries a x16 pre-scale for its fp8 split); partition_broadcast on
    Pool.
  - All projection bias adds are folded into the PSUM evacuation
    (scalar_tensor_tensor with descale scalar + partition-broadcast bias)
    on DVE/Pool -- no bias matmuls on PE.
  - Sub-chunk processing order is sc0, sc1, sc3, sc2: the final
    ReduceScatter is gated by the last chunk's projection, so the smallest
    remaining causal triangle (sc2) goes last; its projection stays f16
    (reads oT directly, skipping the fp8 hi/lo split latency) to keep the
    tail chain short. PSUM is split into 2x[128,1024] S-tile buffers and
    2x[128,512] buffers for QKV/proj units so fillers never contend with
    the attention pipeline; PE warms its p-state on dummy matmuls during
    the DMA-bound preamble.
"""

import os

import numpy as np
import ml_dtypes

import concourse.bacc as bacc
import concourse.mybir as mybir
import concourse.tile as tile
from concourse.bass_utils import run_bass_kernel_spmd

DEBUG = bool(int(os.environ.get("KERNEL_DEBUG", "0")))
PHASE_MARKS = []  # (label, first_instruction_index) for trace attribution

F32 = mybir.dt.float32
F16 = mybir.dt.float16
F8 = mybir.dt.float8e4
E4 = ml_dtypes.float8_e4m3
DR = mybir.MatmulPerfMode.DoubleRow

B, T, C, H = 2, 2048, 1024, 16
HPC = 4                 # heads per core
HD = 64                 # head dim
CG = HPC * 3 * HD       # 768 qkv cols per core
KC = 8                  # contraction chunks (128 channels each)
TT = T // 128           # 16 k tiles
NSC = T // 512          # 4 q sub-chunks
N_CORES = 8
EXP_SCALE = 0.125

XS = 8.0                # host pre-scale on x
WS = 64.0               # host pre-scale on Wqkv / Wproj
OS = 16.0               # device pre-scale on attention output (via recip)
QKV_DESCALE = 1.0 / (XS * WS)
PROJ8_DESCALE = 1.0 / (OS * WS)
PROJ16_DESCALE = 1.0 / OS

# reduce-scatter groups as (row_start, row_end) in COMPLETION order
# (sc0+sc1, then sc3, then sc2 last); each core keeps len/4 rows
RS_GROUPS = [(0, 1024), (1536, 2048), (1024, 1536)]


def _build():
    nc = bacc.Bacc(None, target_bir_lowering=False)

    # x8: [p, w(4), kc(8), e(hi,lo), t(512)] fp8
    x8_in = nc.dram_tensor("x8", [128, 4 * KC * 2 * 512], F8, kind="ExternalInput")
    # w8: [p, kc(8), e(lo,hi), m(768)] fp8
    w8_in = nc.dram_tensor("w8", [128, KC * 2 * CG], F8, kind="ExternalInput")
    qkb_in = nc.dram_tensor("qkb", [128, 4], F32, kind="ExternalInput")
    vb_in = nc.dram_tensor("vb", [1, 256], F16, kind="ExternalInput")
    # wpa8: [p, pair(2), e(lo,hi), c(1024)] fp8 (x64 scaled Wproj rows)
    wpa8_in = nc.dram_tensor("wpa8", [128, 2 * 2 * C], F8, kind="ExternalInput")
    # wpa16: [p, pair(2)*c] f16 unscaled Wproj rows (tail path)
    wpa16_in = nc.dram_tensor("wpa16", [128, 2 * C], F16, kind="ExternalInput")
    # bpr blocks along free dim: [bproj | bproj*OS*WS | bproj*OS]
    # (blocks 1,2 are added in PSUM via a K=1 matmul before a scaled ACT evac)
    bpr_in = nc.dram_tensor("bpr", [1, 3 * C], F16, kind="ExternalInput")
    out_part = nc.dram_tensor("out_part", [T // 4, C], F16, kind="ExternalOutput")

    partial_d = nc.dram_tensor("partial_d", [T, C], F16)
    dbg = {}
    if DEBUG:
        dbg["oT1"] = nc.dram_tensor("dbg_oT1", [128, 2 * 512], F16, kind="ExternalOutput")
        dbg["oT81"] = nc.dram_tensor("dbg_oT81", [128, 4 * 512], F8, kind="ExternalOutput")
        dbg["partial"] = nc.dram_tensor("dbg_partial", [T, C], F16, kind="ExternalOutput")
        dbg["qkT"] = nc.dram_tensor("dbg_qkT", [128, 4 * T], F16, kind="ExternalOutput")
    rsout_d = [
        nc.dram_tensor(f"rsout_d{i}", [(r1 - r0) // 4, C], F16)
        for i, (r0, r1) in enumerate(RS_GROUPS)
    ]

    with tile.TileContext(nc) as tc:
        with (
            tc.tile_pool(name="cpool", bufs=1) as cpool,
            tc.tile_pool(name="main", bufs=1) as main,
            tc.tile_pool(name="stage", bufs=1) as stage,
            tc.tile_pool(name="ps", bufs=1, space="PSUM") as ps,
        ):
            # ---------------- constants ----------------
            vb_bc = cpool.tile([128, 256], F16)
            bias_bc = cpool.tile([128, C], F16)
            ones_col = cpool.tile([1, 128], F16)
            nc.vector.memset(ones_col[:], 1.0)
            # mask stationary: mstat[f, p] = -200 where p > f else 0
            mstat = cpool.tile([128, 128], F16)
            nc.gpsimd.memset(mstat[:], -200.0)
            nc.gpsimd.affine_select(
                out=mstat[:], in_=mstat[:],
                compare_op=mybir.AluOpType.is_ge, fill=0.0,
                base=-1, pattern=[[1, 128]], channel_multiplier=-1,
            )
            # mask moving: identity
            mmov = cpool.tile([128, 128], F16)
            nc.gpsimd.memset(mmov[:], 0.0)
            nc.gpsimd.affine_select(
                out=mmov[:], in_=mmov[:],
                compare_op=mybir.AluOpType.not_equal, fill=1.0,
                base=0, pattern=[[-1, 128]], channel_multiplier=1,
            )

            # ---------------- persistent tensors ----------------
            x8 = main.tile([128, 4 * KC * 2 * 512], F8)
            w8 = main.tile([128, KC * 2 * CG], F8)
            qkb = main.tile([128, 4], F32)
            vb = main.tile([1, 256], F16)
            wpa8 = main.tile([128, 2 * 2 * C], F8)
            wpa16 = main.tile([128, 2 * C], F16)
            bpr = main.tile([1, 3 * C], F16)
            qkT = main.tile([128, 4 * T], F16)             # [Q01;Q23;K01;K23] x T
            v_aug = main.tile([128, TT * HPC * 65], F16)   # per (tt,h): 64 V + ones col
            # f16 attention outs, [pair][512] col layout (x16 pre-scaled);
            # head h lives at partitions 64*(h%2).., column block (h//2)*512
            oT_sb = [
                main.tile([128, 2 * 512], F16, name=f"oT_sb{i}") for i in range(2)
            ]
            # fp8 hi/lo split of oT for the fp8 projection path:
            # [p, pair(2), e(hi,lo), 512]
            oT8_sb = [
                main.tile([128, 2 * 2 * 512], F8, name=f"oT8_sb{i}") for i in range(2)
            ]

            nc.vector.memset(v_aug[:], 1.0)  # ones columns give softmax rowsums

            # ---------------- views ----------------
            w8_r = w8[:].rearrange("p (kc e m) -> p kc e m", kc=KC, e=2)
            w8_in_r = w8_in[:].rearrange("p (kc e m) -> p kc e m", kc=KC, e=2)
            wpa8_r = wpa8[:].rearrange("p (pr e c) -> p pr e c", pr=2, e=2)
            oT8_r = [
                t[:].rearrange("p (pr e c) -> p pr e c", pr=2, e=2) for t in oT8_sb
            ]

            def x8_w(w):
                # [128, kc, e, 512] view of window w
                return x8[:, w * 8192 : (w + 1) * 8192].rearrange(
                    "p (kc e t) -> p kc e t", kc=KC, e=2
                )

            # ---------------- input DMAs ----------------
            # DMA transfers serialize on the DMA-engine pool, so order them
            # by what gates compute: x window 0 + QK weights interleaved per
            # kc pair (attention sc0's S matmuls are the longest dependency
            # chain), V weights next (PV trails S by ~1.5us), then the later
            # x windows.
            for kp in range(4):
                nc.sync.dma_start(
                    x8[:, kp * 2048 : (kp + 1) * 2048],
                    x8_in[:, kp * 2048 : (kp + 1) * 2048],
                )
                nc.sync.dma_start(
                    w8_r[:, 2 * kp : 2 * kp + 2, :, 0:512],
                    w8_in_r[:, 2 * kp : 2 * kp + 2, :, 0:512],
                )
            nc.sync.dma_start(vb[:], vb_in[:])
            nc.sync.dma_start(qkb[:], qkb_in[:])
            nc.gpsimd.partition_broadcast(vb_bc[:], vb[:])
            nc.sync.dma_start(w8_r[:, 0:4, :, 512:768], w8_in_r[:, 0:4, :, 512:768])
            nc.sync.dma_start(w8_r[:, 4:8, :, 512:768], w8_in_r[:, 4:8, :, 512:768])
            nc.sync.dma_start(x8[:, 8192:16384], x8_in[:, 8192:16384])      # w1
            nc.scalar.dma_start(wpa8[:], wpa8_in[:])
            nc.scalar.dma_start(bpr[:], bpr_in[:])
            nc.gpsimd.partition_broadcast(bias_bc[:], bpr[:, 0:C])
            nc.sync.dma_start(x8[:, 16384:24576], x8_in[:, 16384:24576])    # w2
            nc.sync.dma_start(x8[:, 24576:32768], x8_in[:, 24576:32768])    # w3
            nc.scalar.dma_start(wpa16[:], wpa16_in[:])

            # ---------------- emit helpers ----------------
            def mm_pairs(pp_slice, stat_of, mov_of):
                """Emit the compensated fp8 matmul group: for each kc pair,
                hi*hi (2 slots = the 2 k-tiles), then a cross matmul per
                k-tile (slots = hi*lo + lo*hi)."""
                n = KC // 2
                for p in range(n):
                    c0 = 2 * p
                    nc.tensor.matmul(
                        pp_slice, stat_of(c0, None), mov_of(c0, None),
                        start=(p == 0), stop=False, perf_mode=DR,
                    )
                    for c in (c0, c0 + 1):
                        nc.tensor.matmul(
                            pp_slice, stat_of(None, c), mov_of(None, c),
                            start=False, stop=(c == KC - 1), perf_mode=DR,
                        )

            def emit_v(tt):
                PHASE_MARKS.append((f"V{tt}", len(nc.inst_map)))
                w, tloc = divmod(tt, 4)
                xw = x8_w(w)
                tr = slice(tloc * 128, (tloc + 1) * 128)
                pp = ps.tile([128, 512], F32, tag="pmm", bufs=2)

                def stat(pair_c0, cross_c):
                    if pair_c0 is not None:  # hi*hi: slots (kc0,hi),(kc1,hi)
                        return xw[:, pair_c0 : pair_c0 + 2, 0, tr]
                    return xw[:, cross_c, :, tr]  # (hi,lo)

                def mov(pair_c0, cross_c):
                    if pair_c0 is not None:  # slots (kc0,hi),(kc1,hi)
                        return w8_r[:, pair_c0 : pair_c0 + 2, 1, 512:768]
                    return w8_r[:, cross_c, :, 512:768]  # (lo,hi)

                mm_pairs(pp[:, 0:256], stat, mov)
                vt = v_aug[:, tt * HPC * 65 : (tt + 1) * HPC * 65].rearrange(
                    "p (h c) -> p h c", c=65
                )[:, :, 0:64]
                nc.vector.scalar_tensor_tensor(
                    out=vt,
                    in0=pp[:, 0:256].rearrange("p (h c) -> p h c", c=64),
                    scalar=QKV_DESCALE,
                    in1=vb_bc[:].rearrange("p (h c) -> p h c", c=64),
                    op0=mybir.AluOpType.mult,
                    op1=mybir.AluOpType.add,
                )

            def emit_qk(i, tch):
                PHASE_MARKS.append((f"QK({i},{tch})", len(nc.inst_map)))
                xw = x8_w(tch)
                ir = slice(i * 128, (i + 1) * 128)
                pp0 = ps.tile([128, 512], F32, tag="pmm", bufs=2)
                pp = pp0[:]

                def stat(pair_c0, cross_c):
                    if pair_c0 is not None:
                        return w8_r[:, pair_c0 : pair_c0 + 2, 1, ir]
                    return w8_r[:, cross_c, :, ir]

                def mov(pair_c0, cross_c):
                    if pair_c0 is not None:
                        return xw[:, pair_c0 : pair_c0 + 2, 0, :]
                    return xw[:, cross_c, :, :]

                mm_pairs(pp, stat, mov)
                dst = qkT[:, i * T + tch * 512 : i * T + (tch + 1) * 512]
                # DVE, not ACT: evacuations on ACT would queue ahead of the
                # attention exps and stretch the S-tile free latency
                nc.vector.tensor_scalar(
                    out=dst, in0=pp, scalar1=QKV_DESCALE,
                    scalar2=qkb[:, i : i + 1],
                    op0=mybir.AluOpType.mult, op1=mybir.AluOpType.add,
                )

            def emit_qk2(ia, ib, tch):
                # two QK units interleaved by kc pair: during the DMA-bound
                # preamble neither unit serializes behind the other's stall,
                # so both evacuate as soon as the last kc pieces land
                PHASE_MARKS.append((f"QK2({ia},{ib},{tch})", len(nc.inst_map)))
                xw = x8_w(tch)
                pps = {}
                for i in (ia, ib):
                    pps[i] = ps.tile(
                        [128, 512], F32, tag="pmm", bufs=2, name=f"qk2_{i}"
                    )
                for p in range(KC // 2):
                    c0 = 2 * p
                    for i in (ia, ib):
                        ir = slice(i * 128, (i + 1) * 128)
                        nc.tensor.matmul(
                            pps[i][:], w8_r[:, c0 : c0 + 2, 1, ir],
                            xw[:, c0 : c0 + 2, 0, :],
                            start=(p == 0), stop=False, perf_mode=DR,
                        )
                        for c in (c0, c0 + 1):
                            nc.tensor.matmul(
                                pps[i][:], w8_r[:, c, :, ir], xw[:, c, :, :],
                                start=False, stop=(c == KC - 1), perf_mode=DR,
                            )
                for i in (ia, ib):
                    dst = qkT[:, i * T + tch * 512 : i * T + (tch + 1) * 512]
                    nc.vector.tensor_scalar(
                        out=dst, in0=pps[i][:], scalar1=QKV_DESCALE,
                        scalar2=qkb[:, i : i + 1],
                        op0=mybir.AluOpType.mult, op1=mybir.AluOpType.add,
                    )

            def emit_att_head(sc, h, fillers=None, make_fp8=True,
                              pmm_first=False):
                PHASE_MARKS.append((f"att{sc}h{h}", len(nc.inst_map)))
                qT = qkT[64 * (h % 2) : 64 * (h % 2) + 64, (h // 2) * T : (h // 2 + 1) * T]
                kT = qkT[64 * (h % 2) : 64 * (h % 2) + 64, (2 + h // 2) * T : (3 + h // 2) * T]
                oT_ps = ps.tile([65, 512], F32, tag="ot", bufs=2)
                n_kj = (sc + 1) * 4
                npairs = n_kj // 2

                def pair_layout(p):
                    # [(bank_off, q_off, cols, kj), ...]; pack both k tiles
                    # into one PSUM bank when their columns fit (saves exp
                    # span and a bank)
                    kj0, kj1 = 2 * p, 2 * p + 1
                    qo0 = max(0, kj0 * 128 - sc * 512)
                    qo1 = max(0, kj1 * 128 - sc * 512)
                    c0, c1 = 512 - qo0, 512 - qo1
                    if c0 + c1 <= 512:
                        return [(0, qo0, c0, kj0), (c0, qo1, c1, kj1)], c0 + c1
                    return [(0, qo0, c0, kj0), (512, qo1, c1, kj1)], 512 + c1

                def emit_s_pair(p):
                    layout, span = pair_layout(p)
                    one_bank = layout[1][0] < 512
                    # heads with no fillers: first pair via two pmm
                    # half-tiles (idle there) so the head start skips the
                    # previous head's trailing-exp smm WAR
                    if p == 0 and npairs >= 5 and (pmm_first or not fillers):
                        pt = stage.tile([128, 1024], F16, tag="pt", bufs=4)
                        for idx, (boff, q_off, cols, kj) in enumerate(layout):
                            s5 = ps.tile(
                                [128, 512], F32, tag="pmm", bufs=2,
                                name=f"s5_{idx}",
                            )
                            nc.tensor.matmul(
                                s5[:, 0:cols],
                                kT[:, kj * 128 : (kj + 1) * 128],
                                qT[:, sc * 512 + q_off : (sc + 1) * 512],
                                start=True, stop=True,
                            )
                            nc.scalar.activation(
                                pt[:, boff : boff + cols], s5[:, 0:cols],
                                mybir.ActivationFunctionType.Exp,
                                scale=EXP_SCALE,
                            )
                        return pt, layout
                    st = ps.tile([128, 1024], F32, tag="smm", bufs=2)
                    pt = stage.tile([128, 1024], F16, tag="pt", bufs=4)
                    # per-PSUM-bank balanced start/stop: in the one_bank pack
                    # both k-tiles share a group (start zeroes the whole zero
                    # region, so the second k-tile's disjoint columns
                    # accumulate onto zeros); otherwise one group per bank
                    for idx, (boff, q_off, cols, kj) in enumerate(layout):
                        diag = kj >= sc * 4
                        first = idx == 0 or not one_bank
                        last_in_group = (not one_bank) or idx == 1
                        nc.tensor.matmul(
                            st[:, boff : boff + cols],
                            kT[:, kj * 128 : (kj + 1) * 128],
                            qT[:, sc * 512 + q_off : (sc + 1) * 512],
                            start=first,
                            stop=(not diag) and last_in_group,
                        )
                        if diag:
                            nc.tensor.matmul(
                                st[:, boff : boff + 128], mstat[:], mmov[:],
                                start=False, stop=last_in_group,
                            )
                    # one exp covering both halves (cols between valid ranges
                    # hold stale PSUM; the pt garbage there is never read)
                    nc.scalar.activation(
                        pt[:, :span], st[:, :span],
                        mybir.ActivationFunctionType.Exp,
                        scale=EXP_SCALE,
                    )
                    return pt, layout

                def emit_pv_pair(p, pt, layout):
                    for boff, q_off, cols, kj in layout:
                        vv = v_aug[:, (kj * HPC + h) * 65 : (kj * HPC + h + 1) * 65]
                        nc.tensor.matmul(
                            oT_ps[:, q_off:512],
                            vv,
                            pt[:, boff : boff + cols],
                            start=(kj == 0),
                            stop=(kj == n_kj - 1),
                        )

                # software pipeline: emit S(p+1) before PV(p) so PE always has
                # matmul work queued while exp(p) completes on ACT; fillers
                # (independent work units, one LIST per exp-wait slot) absorb
                # ACT-bound gaps; leftovers flush before the final PV so they
                # still precede the normalize chain
                fillers = [list(f) for f in (fillers or [])]
                pend = emit_s_pair(0)
                for p in range(1, npairs):
                    nxt = emit_s_pair(p)
                    if fillers:
                        for f in fillers.pop(0):
                            f()
                    emit_pv_pair(p - 1, *pend)
                    pend = nxt
                for fl in fillers:
                    for f in fl:
                        f()
                emit_pv_pair(npairs - 1, *pend)
                # normalize chain for this head (DVE + Pool), frees oT_ps;
                # rowsum scaled by 1/OS so oT carries a xOS pre-scale
                rs = stage.tile([1, 512], F32, tag="rs", bufs=4)
                recip = stage.tile([1, 512], F32, tag="recip", bufs=4)
                bc = stage.tile([64, 512], F32, tag="bc", bufs=4)
                if sc == 2 and h == 1:
                    # last head: pipeline the rowsum->recip->broadcast chain
                    # in column halves across ACT/DVE/Pool so the first fin
                    # closes unblock ~0.5us earlier
                    for hf in range(2):
                        hs = slice(hf * 256, (hf + 1) * 256)
                        nc.scalar.activation(
                            rs[:, hs], oT_ps[64:65, hs],
                            mybir.ActivationFunctionType.Copy, scale=1.0 / OS,
                        )
                        nc.vector.reciprocal_approx_fast(recip[:, hs], rs[:, hs])
                        nc.gpsimd.partition_broadcast(bc[:, hs], recip[:, hs])
                else:
                    nc.vector.tensor_scalar_mul(rs[:], oT_ps[64:65, :], 1.0 / OS)
                    nc.vector.reciprocal_approx_fast(recip[:], rs[:])
                    nc.gpsimd.partition_broadcast(bc[:], recip[:])
                o16 = oT_sb[sc % 2][
                    64 * (h % 2) : 64 * (h % 2) + 64,
                    (h // 2) * 512 : (h // 2 + 1) * 512,
                ]
                if sc == 2 and h == 1:
                    # last head: 128-col pieces so each fin close piece j
                    # unblocks as soon as its oT columns land
                    for q4 in range(4):
                        qs = slice(q4 * 128, (q4 + 1) * 128)
                        nc.vector.tensor_mul(
                            o16[:, qs], oT_ps[0:64, qs], bc[:, qs]
                        )
                else:
                    nc.vector.tensor_mul(o16, oT_ps[0:64, :], bc[:])
                if make_fp8:
                    # both on Pool: keeps the DVE queue clear for the next
                    # head's rs/recip/mul chain (Pool has slack and its
                    # in-queue delay hides under the next head's PV)
                    pr = 64 * (h % 2)
                    hi = oT8_r[sc % 2][pr : pr + 64, h // 2, 0, :]
                    lo = oT8_r[sc % 2][pr : pr + 64, h // 2, 1, :]
                    nc.gpsimd.tensor_copy(hi, o16)
                    nc.gpsimd.tensor_sub(lo, o16, hi)

            def emit_proj_piece(sc, j, evac_on_act=None, dmaq=None):
                # fp8 path: project rows [sc*512 + j*128, +128).
                # DVE evac: stt folds descale + bias. ACT evac: bias enters
                # PSUM via a K=1 ones x (bias/descale) matmul, then a scaled
                # Copy (Pool cannot read PSUM).
                PHASE_MARKS.append((f"proj({sc},{j})", len(nc.inst_map)))
                r0 = sc * 512 + j * 128
                o8 = oT8_r[sc % 2]
                jr = slice(j * 128, (j + 1) * 128)
                on_act = bool(evac_on_act)
                pst = stage.tile([128, 1024], F16, tag="pst", bufs=4)
                for nch in range(2):
                    nr = slice(nch * 512, (nch + 1) * 512)
                    pp = ps.tile([128, 512], F32, tag="pmm", bufs=2)
                    nc.tensor.matmul(
                        pp[:],
                        o8[:, :, 0, jr],        # (pair0 hi, pair1 hi)
                        wpa8_r[:, :, 1, nr],    # (pair0 hi, pair1 hi)
                        start=True, stop=False, perf_mode=DR,
                    )
                    if on_act:
                        nc.tensor.matmul(
                            pp[:], ones_col[:],
                            bpr[:, C + nch * 512 : C + (nch + 1) * 512],
                            start=False, stop=False,
                        )
                    for p in range(2):
                        nc.tensor.matmul(
                            pp[:],
                            o8[:, p, :, jr],      # (hi, lo)
                            wpa8_r[:, p, :, nr],  # (lo, hi)
                            start=False, stop=(p == 1), perf_mode=DR,
                        )
                    if on_act:
                        nc.scalar.activation(
                            pst[:, nr], pp[:], mybir.ActivationFunctionType.Copy,
                            scale=PROJ8_DESCALE,
                        )
                    else:
                        nc.vector.scalar_tensor_tensor(
                            out=pst[:, nr], in0=pp[:], scalar=PROJ8_DESCALE,
                            in1=bias_bc[:, nr],
                            op0=mybir.AluOpType.mult, op1=mybir.AluOpType.add,
                        )
                # spread partial-write issues across SEQ queues: each DMA
                # issue occupies its queue ~650ns and the tail needs several
                # in flight at once
                dq = dmaq if dmaq is not None else [nc.sync, nc.scalar][j % 2]
                dq.dma_start(partial_d[r0 : r0 + 128, :], pst[:])

            def emit_rs(gi):
                r0, r1 = RS_GROUPS[gi]
                nc.gpsimd.collective_compute(
                    "ReduceScatter",
                    mybir.AluOpType.add,
                    replica_groups=[[0, 1, 2, 3], [4, 5, 6, 7]],
                    ins=[partial_d[r0:r1, :]],
                    outs=[rsout_d[gi][:]],
                )

            def emit_out_copy(gi):
                # deferred to the tail: an out-copy waiting on its collective
                # must not sit in an in-order DMA queue ahead of partial
                # writes that later collectives depend on
                r0, r1 = RS_GROUPS[gi]
                og = sum((b1 - b0) // 4 for (b0, b1) in RS_GROUPS[:gi])
                ln4 = (r1 - r0) // 4
                # huge scheduling-floor: keeps the tile scheduler from
                # hoisting these RS-gated copies ahead of the tail partial
                # writes in the same queue (ordering hint only, no HW wait)
                with tc.tile_wait_until(1.0):
                    nc.sync.dma_start(out_part[og : og + ln4, :], rsout_d[gi][:])

            # ---------------- schedule ----------------
            # PE warmup: the tensor engine clock ramps with sustained use and
            # the first ~9us are DMA-bound anyway, so spin dependency-free
            # dummy matmuls to hit full p-state before real work arrives.
            warm = ps.tile([128, 1024], F32, tag="smm", bufs=2)
            for i in range(120):
                nc.tensor.matmul(
                    warm[:, (i % 8) * 128 : (i % 8 + 1) * 128],
                    mstat[:], mmov[:],
                    start=True, stop=True, skip_group_check=True,
                )

            # Sub-chunk order: sc0, sc1, sc3, sc2. The LAST chunk's
            # projection gates the final ReduceScatter, so the smallest
            # remaining triangle (sc2) goes last; fin2 is the f16 tail path.
            # fp8 matmuls outrun the input DMAs at the start, so only the
            # QK(tch0) units precede att0 and everything else fills
            # attention's exp-wait slots in DMA-arrival order.
            emit_qk2(1, 3, 0)
            emit_qk2(0, 2, 0)

            # att sc0; V(0..3) land between S and PV; everything att1
            # needs fills the rest of att0's exp-wait slots
            att0_fill = [
                [[lambda: emit_v(0), lambda: emit_v(1)],
                 [lambda: emit_v(2), lambda: emit_v(3)]],
                [[lambda: emit_v(4)], [lambda: emit_v(5)]],
                [[lambda: emit_v(6)], [lambda: emit_qk(1, 1)]],
                [[lambda: emit_v(7)], [lambda: emit_qk(3, 1)]],
            ]
            for k, h in enumerate((2, 3, 0, 1)):
                emit_att_head(0, h, att0_fill[k])

            # att sc1; x-window-2/3 units + sc0 proj (oT8[0] is stable from
            # here on) fill the slots
            att1_fill = [
                [[lambda: emit_qk(0, 1)], [lambda: emit_v(8)],
                 [lambda: emit_v(9)]],
                [[lambda: emit_qk(2, 1)], [lambda: emit_v(10)],
                 [lambda: emit_v(11)]],
                [[lambda: emit_proj_piece(0, 0)], [lambda: emit_v(12)],
                 [lambda: emit_v(13)]],
                [[lambda: emit_proj_piece(0, 1)], [lambda: emit_v(14)],
                 [lambda: emit_v(15)]],
            ]
            for k, h in enumerate((2, 3, 0, 1)):
                emit_att_head(1, h, att1_fill[k])

            # fin1: att3's own Q/K units lead (covering the last sc1 head's
            # normalize chain), then the sc1 projection (it reads oT8[1],
            # which att3 overwrites -- the WAR dep keeps the reads safe)
            emit_qk(1, 3)
            emit_qk(3, 3)
            emit_qk(3, 2)
            emit_proj_piece(1, 0)
            emit_proj_piece(1, 1)
            emit_proj_piece(1, 2)
            emit_proj_piece(1, 3)

            # att sc3 (biggest triangle): remaining K/Q units for its own
            # later heads + sc0's remaining proj (oT8[0] stays untouched)
            att3_fill = [
                [[lambda: emit_qk(2, 2)], [lambda: emit_qk(2, 3)],
                 [lambda: emit_qk(0, 3)]],
                [[lambda: emit_qk(0, 2)], [lambda: emit_proj_piece(0, 2)]],
                [[lambda: emit_proj_piece(0, 3)]],
                [[lambda: emit_rs(0)]],
            ]
            for k, h in enumerate((2, 3, 0, 1)):
                emit_att_head(3, h, att3_fill[k], pmm_first=(k == 3))
            # fin3 lead-in: one ready QK unit covers the last sc3 head's
            # chain, then the first sc3 proj pieces
            emit_qk(1, 2)
            emit_proj_piece(3, 0)
            emit_proj_piece(3, 1)

            # att sc2 (last): rest of fin3 + its rs early; sc2 skips the fp8
            # split (f16 tail path).
            att2_fill = {
                2: [[lambda: emit_proj_piece(3, 2)]],
                3: [[lambda: emit_proj_piece(3, 3)]],
                0: [[lambda: emit_rs(1)]],
            }
            for k, h in enumerate((2, 3, 0, 1)):
                emit_att_head(2, h, att2_fill.get(h), make_fp8=False,
                              pmm_first=(k > 0))

            # fin2: f16 tail projection for sc2 (no bias matmuls on the DVE
            # pieces; bias + 1/OS descale folded into the stt evacuation)
            oT_cur = oT_sb[0]
            tail_q = [nc.gpsimd, nc.scalar, nc.sync, nc.sync]
            tail_act = [True, True, False, False]

            def fin_open(j, kind):
                # kind: "smm" -> one [128,1024] tile split in halves;
                # "pmm" -> two [128,512] tiles (attention is done, both
                # pools are free -- this lets 3 opens stay in flight)
                PHASE_MARKS.append((f"fin_open{j}", len(nc.inst_map)))
                if kind == "smm":
                    t = ps.tile([128, 1024], F32, tag="smm", bufs=2)
                    aps = [t[:, 0:512], t[:, 512:1024]]
                else:
                    aps = []
                    for _n in range(2):
                        fpp = ps.tile(
                            [128, 512], F32, tag="pmm", bufs=2, name=f"fpp{_n}"
                        )
                        aps.append(fpp[:])
                for nch in range(2):
                    nc.tensor.matmul(
                        aps[nch],
                        oT_cur[:, 512 + j * 128 : 512 + (j + 1) * 128],
                        wpa16[:, C + nch * 512 : C + (nch + 1) * 512],
                        start=True, stop=False,
                    )
                    if tail_act[j]:
                        nc.tensor.matmul(
                            aps[nch],
                            ones_col[:],
                            bpr[:, 2 * C + nch * 512 : 2 * C + (nch + 1) * 512],
                            start=False, stop=False,
                        )
                return aps

            def fin_close(j, aps):
                PHASE_MARKS.append((f"fin_close{j}", len(nc.inst_map)))
                for nch in range(2):
                    nc.tensor.matmul(
                        aps[nch],
                        oT_cur[:, j * 128 : (j + 1) * 128],
                        wpa16[:, nch * 512 : (nch + 1) * 512],
                        start=False, stop=True,
                    )
                pst = stage.tile([128, 1024], F16, tag="pst", bufs=4)
                for nch in range(2):
                    nr = slice(nch * 512, (nch + 1) * 512)
                    if tail_act[j]:
                        nc.scalar.activation(
                            pst[:, nr], aps[nch],
                            mybir.ActivationFunctionType.Copy,
                            scale=PROJ16_DESCALE,
                        )
                    else:
                        nc.vector.scalar_tensor_tensor(
                            out=pst[:, nr], in0=aps[nch], scalar=PROJ16_DESCALE,
                            in1=bias_bc[:, nr],
                            op0=mybir.AluOpType.mult, op1=mybir.AluOpType.add,
                        )
                r0 = 2 * 512 + j * 128
                tail_q[j].dma_start(partial_d[r0 : r0 + 128, :], pst[:])

            if DEBUG:
                nc.sync.dma_start(dbg["oT1"][:], oT_sb[1][:])
                nc.sync.dma_start(dbg["oT81"][:], oT8_sb[1][:])
                nc.sync.dma_start(dbg["qkT"][:], qkT[:])
            pps = [fin_open(0, "smm"), fin_open(1, "smm"), fin_open(2, "pmm")]
            fin_close(0, pps[0])
            pp3 = fin_open(3, "smm")
            fin_close(1, pps[1])
            fin_close(2, pps[2])
            fin_close(3, pp3)
            emit_rs(2)
            for gi in range(len(RS_GROUPS)):
                emit_out_copy(gi)
            if DEBUG:
                nc.sync.dma_start(dbg["partial"][:], partial_d[:])

    nc.finalize()
    return nc


_NC = None


def _get_nc():
    global _NC
    if _NC is None:
        _NC = _build()
    return _NC


def _perm_qkv(w):
    # (..., h*192 + t*64 + c) -> (..., t*256 + h*64 + c)
    s = w.shape[:-1]
    return np.ascontiguousarray(
        w.reshape(*s, HPC, 3, HD).swapaxes(-3, -2).reshape(*s, CG)
    )


def _hilo(a):
    hi = a.astype(E4)
    lo = (a - hi.astype(np.float32)).astype(E4)
    return hi, lo


def _make_in_maps(x, Wqkv, bqkv, Wproj, bproj):
    x = np.asarray(x, dtype=np.float32)
    Wqkv = np.asarray(Wqkv, dtype=np.float32)
    bqkv = np.asarray(bqkv, dtype=np.float32)
    Wproj = np.asarray(Wproj, dtype=np.float32)
    bproj = np.asarray(bproj, dtype=np.float32)

    in_maps = []
    for c in range(N_CORES):
        b, g = divmod(c, 4)
        # x8: [p, w, kc, e(hi,lo), t]
        xT = x[b].T * XS  # (C, T)
        xa = xT.reshape(KC, 128, 4, 512).transpose(1, 2, 0, 3)  # [p, w, kc, t]
        xh, xl = _hilo(xa)
        x8 = np.stack([xh, xl], axis=3).reshape(128, -1)
        # w8: [p, kc, e(lo,hi), m]
        wp_ = _perm_qkv(Wqkv[:, g * CG : (g + 1) * CG]) * WS
        wa = wp_.reshape(KC, 128, CG).transpose(1, 0, 2)  # [p, kc, m]
        wh, wl = _hilo(wa)
        w8 = np.stack([wl, wh], axis=2).reshape(128, -1)
        bq = _perm_qkv(bqkv[g * CG : (g + 1) * CG])
        qkb = np.ascontiguousarray(bq[:512].reshape(4, 128).T).astype(np.float32)
        vb = bq[512:768].reshape(1, 256).astype(np.float16)
        # wpa: [p, pair, c]
        wpa = np.zeros((128, 2, C), np.float32)
        for pair in range(2):
            wpa[:, pair] = Wproj[
                g * 256 + pair * 128 : g * 256 + (pair + 1) * 128, :
            ]
        ph, pl = _hilo(wpa * WS)
        wpa8 = np.stack([pl, ph], axis=2).reshape(128, -1)
        b0 = bproj if g == 0 else np.zeros(C, np.float32)
        bpr = np.concatenate([b0, b0 * OS * WS, b0 * OS]).reshape(1, 3 * C)
        in_maps.append(
            {
                "x8": x8,
                "w8": w8,
                "qkb": qkb,
                "vb": vb,
                "wpa8": wpa8,
                "wpa16": wpa.reshape(128, -1).astype(np.float16),
                "bpr": bpr.astype(np.float16),
            }
        )
    return in_maps


def _run(in_maps, trace=False):
    nc = _get_nc()
    return run_bass_kernel_spmd(nc, in_maps, list(range(N_CORES)), trace=trace)


def kernel(x, Wqkv, bqkv, Wproj, bproj):
    in_maps = _make_in_maps(x, Wqkv, bqkv, Wproj, bproj)
    res = _run(in_maps)
    out = np.empty((B, T, C), np.float32)
    for c in range(N_CORES):
        b, g = divmod(c, 4)
        op = res.results[c]["out_part"].astype(np.float32)
        og = 0
        for r0, r1 in RS_GROUPS:
            ln4 = (r1 - r0) // 4
            out[b, r0 + g * ln4 : r0 + (g + 1) * ln4, :] = op[og : og + ln4]
            og += ln4
    return out
